# revision 60
# baseline (speedup 1.0000x reference)
"""Trainium2 Bass kernel for the FermiNet-style single-configuration ansatz.

Computes log|psi| = logdet(orb_u) + logdet(orb_d) for one electron
configuration. The whole forward runs replicated on 8 NeuronCores (the
problem is tiny; inter-core collectives have a ~7-20us latency floor that
dwarfs the ~1 GFLOP of compute, so replication is the fastest correct
distribution) and core 0's scalar output is returned.

Layout choices (see inline comments):
  - p-tensor kept transposed+doubled: pT2[q, j*64 + i_local], q<64 = feature g
    for spin-up electrons (i<64), q>=64 = feature g for spin-down. This makes
    the per-pair feature matmul a K=64 contraction over partitions, lets
    spin-up/down run concurrently in separate PE array quadrants
    (tile_position), and makes the i-mean a free-dim segmented reduce.
  - residuals p2 = t2 + t1 are never materialized; the matmul and the means
    distribute over the sum (tanh outputs t_l are kept separately).
  - p-mean contributions to the s-layers use ONE cumulative [128,128] tile
    (DVE adds between layers) instead of repeated Vw chunks.
  - s-layer weights are all prefetched to SBUF right after the p-tensor
    build (descriptor-striped big DMAs), so s-layer matmuls never stall.
  - biases along the free dim are added as rank-1 (ones x b) matmuls into the
    same PSUM accumulation group.
  - logdet via unpivoted rank-1 Gaussian elimination on the stacked [A_u;A_d]
    [128,64] tile. Per step ONE K=128 one-hot matmul broadcasts the pivot
    row of both halves straight to PSUM, then 3 DVE ops (reciprocal /
    multiplier / fused scalar_tensor_tensor rank-1 update). The one-hot
    lhsT is built just-in-time one step ahead on the otherwise-idle SCALAR
    engine (bias-broadcast trick), so neither PE nor DVE pays for it.
    Unpivoted LU is stable here (growth factor ~700, logdet error ~6e-3 in
    f32 vs 2e-2 rtol). diag(U) is read off the final A at the end.

Hard-won platform notes (cost ~6 HW iterations to learn):
  - matmul PE time scales with OUT FREE SIZE (N) x cycles/row(rhs dtype),
    NOT with K; f32 runs as 2 half-speed passes when M > 64.
  - bulk GPSIMD ucode is poison: consumers of ANY gpsimd output wait for
    the engine's full queue DRAIN (a 16us CC build stalled all DMAs 15us).
  - stride-0 (broadcast_to) DMA APs fail neuronxcc codegen.
  - f32r matmuls require every PRODUCER of their operands to round to
    f32r (BIR verifier), and still hit an ISA check failure here.
  - tensor_tensor_reduce crashes the exec unit on TRN2 HW (sim-only op).
  - act tables: ~6-7 live; warm Abs/Ln AFTER the last Tanh/Identity use
    (pin the warm-up late via a data dependency or the scheduler hoists it).
"""

import numpy as np

import concourse.bass as bass
import concourse.bacc as bacc
import concourse.mybir as mybir
import concourse.tile as tile
from concourse import bass_isa, masks

F32 = mybir.dt.float32
F32R = mybir.dt.float32r
FP16 = mybir.dt.float16
AF = mybir.ActivationFunctionType
Alu = mybir.AluOpType

NE, NA, NSV, NPV, NU = 128, 32, 512, 64, 64

INPUT_SPECS = [
    ("r", (128, 3)), ("a", (32, 3)),
    ("V0_w", (392, 512)), ("V0_b", (512,)),
    ("V1_w", (1664, 512)), ("V1_b", (512,)),
    ("V2_w", (1664, 512)), ("V2_b", (512,)),
    ("W0_w", (4, 64)), ("W0_b", (64,)),
    ("W1_w", (64, 64)), ("W1_b", (64,)),
    ("W2_w", (64, 64)), ("W2_b", (64,)),
    ("after_w", (1664, 512)), ("after_b", (512,)),
    ("vhu_w", (512, 256)), ("vhu_b", (256,)),
    ("vhd_w", (512, 256)), ("vhd_b", (256,)),
    ("wu_w", (256, 64)), ("wu_b", (64,)),
    ("wd_w", (256, 64)), ("wd_b", (64,)),
]


def _r(ap):
    return ap.bitcast(F32R)


def _program(tc, nc, ins, out_d, dbg_d=None):
    import os
    stage = os.environ.get("KSTAGE", "full")
    ctx_pools = {}

    def pool(name, bufs, space="SBUF"):
        if name not in ctx_pools:
            ctx_pools[name] = tc.alloc_tile_pool(name=name, bufs=bufs,
                                                 space=space)
        return ctx_pools[name]

    const = pool("const", 1)
    work = pool("work", 1)
    pipe2 = pool("pipe2", 2)
    sbcast = pool("sbcast", 8)
    svtp = pool("svtp", 4)
    big = pool("big", 1)
    wpre = pool("wpre", 1)
    wstream = pool("wstream", 4)
    lu = pool("lu", 3)
    ps_big = pool("ps_big", 2, space="PSUM")
    ps_sm = pool("ps_sm", 2, space="PSUM")
    ps_lu = pool("ps_lu", 1, space="PSUM")
    ps_s = pool("ps_sx", 1, space="PSUM")

    dma = nc.sync.dma_start

    # ---------------- constants ----------------
    ident = const.tile([128, 128], F32, tag="ident")
    masks.make_identity(nc, ident[:])
    ones_row = const.tile([1, 128], F32, tag="ones_row")
    nc.gpsimd.memset(ones_row[:], 1.0)
    inv64_col = const.tile([128, 1], F32, tag="inv64")
    nc.gpsimd.memset(inv64_col[:], 1.0 / 64.0)

    # LU strict-lower mask (negated): negmask[p, k] = -1 iff (p % 64) > k.
    # affine_select indexes partitions view-relative (probed in sim), so the
    # same base works for both halves.
    negmask = const.tile([128, 64], F32, tag="negmask")
    nc.gpsimd.memset(negmask[:], -1.0)
    for half in range(2):
        nc.gpsimd.affine_select(
            out=negmask[half * 64:(half + 1) * 64, :],
            in_=negmask[half * 64:(half + 1) * 64, :],
            pattern=[[-1, 64]], compare_op=Alu.is_ge,
            fill=0.0, base=-1, channel_multiplier=1)

    # ---------------- geometry ----------------
    r_sb = work.tile([128, 3], F32, tag="r_sb")
    dma(r_sb[:], ins["r"][:])

    # rT4 = [r^T ; ones] as [4, 128]
    psr = ps_sm.tile([4, 128], F32, tag="small")
    nc.tensor.transpose(psr[0:3, :], r_sb[:], ident[:])
    rT4 = const.tile([4, 128], F32, tag="rT4")
    nc.gpsimd.memset(rT4[:], 1.0)  # row 3 stays ones
    nc.vector.tensor_copy(rT4[0:3, :], psr[0:3, :])

    def delta_rows(t, nj, val):
        """t[c, j*3+k] = val*(k == c) for c in 0..2; row 3 zeroed (DMA after).
        Compute-engine APs must start at partition 0/32/64/96, so build the
        delta pattern with one affine_select over all 4 rows. NOTE: keep
        total GPSIMD ucode work tiny -- every consumer of ANY gpsimd output
        waits for the engine's full queue drain."""
        nc.gpsimd.memset(t[:], val)
        nc.gpsimd.affine_select(
            out=t[:], in_=t[:], pattern=[[0, nj], [1, 3]],
            compare_op=Alu.is_equal, fill=0.0, base=0, channel_multiplier=-1)

    def into_row3(t, src_flat, width, tag, scale):
        st = pipe2.tile([1, width], F32, tag=tag)
        dma(st[:], src_flat)
        nc.scalar.mul(st[:], st[:], scale)
        dma(t[3:4, :], st[:])

    # Wra[4, 96]: ra = [r|1] @ Wra,  ra[i, j*3+c] = r[i,c] - a[j,c]
    # ra[i, j] = r[i] - a[j]
    Wra = const.tile([4, 3 * NA], F32, tag="Wra")
    delta_rows(Wra, NA, 1.0)
    into_row3(Wra, ins["a"][:].rearrange("a b -> (a b)"), 3 * NA, "nga", -1.0)

    ps_ra_t = ps_big.tile([128, 1024], F32, tag="big1024")
    ps_ra = ps_ra_t[:, 0:3 * NA]
    nc.tensor.matmul(ps_ra, rT4[:], Wra[:], start=True, stop=True)
    ra_sb = work.tile([128, 3 * NA], F32, tag="ra_sb")
    nc.vector.tensor_copy(ra_sb[:], ps_ra)
    ra2 = work.tile([128, 3 * NA], F32, tag="ra2")
    nc.scalar.square(ra2[:], ps_ra)
    ra_len2 = work.tile([128, NA], F32, tag="ra_len2")
    nc.vector.reduce_sum(
        ra_len2[:], ra2[:].rearrange("p (j c) -> p j c", c=3),
        axis=mybir.AxisListType.X,
    )
    ra_len = work.tile([128, NA], F32, tag="ra_len")
    nc.scalar.sqrt(ra_len[:], ra_len2[:])
    # e_col[i] = sum_j exp(-|r_i - a_j|)
    e_col = const.tile([128, 1], F32, tag="e_col")
    eexp = work.tile([128, NA], F32, tag="eexp")
    nc.scalar.activation(eexp[:], ra_len[:], AF.Exp, scale=-1.0,
                         accum_out=e_col[:])

    def dbg_out(src_ap):
        o = work.tile([1, 1], F32, tag="out_sb")
        nc.scalar.mul(o[:], src_ap, 1.0)
        dma(out_d[:], o[:])

    # s_v0 [128, 128]: interleaved [ra_x, ra_y, ra_z, |ra|] per atom
    s_v0 = work.tile([128, 128], F32, tag="s_v0")
    v4 = s_v0[:].rearrange("p (j k) -> p j k", k=4)
    nc.scalar.activation(v4[:, :, 0:3],
                         ra_sb[:].rearrange("p (j c) -> p j c", c=3),
                         AF.Identity)
    nc.scalar.activation(v4[:, :, 3:4],
                         ra_len[:].rearrange("p (j k) -> p j k", k=1),
                         AF.Identity)

    # rr: Wrr[4, 384], rr = [r|1] @ Wrr, rr[i, j*3+c] = r[i,c] - r[j,c]
    # reference convention: rr[i, j] = r[j] - r[i]
    Wrr = const.tile([4, 3 * NE], F32, tag="Wrr")
    delta_rows(Wrr, NE, -1.0)
    into_row3(Wrr, ins["r"][:].rearrange("a b -> (a b)"), 3 * NE, "ngr", 1.0)

    ps_rr_t = ps_big.tile([128, 1024], F32, tag="big1024")
    ps_rr = ps_rr_t[:, 0:3 * NE]
    nc.tensor.matmul(ps_rr, rT4[:], Wrr[:], start=True, stop=True)
    rr_sb = work.tile([128, 3 * NE], F32, tag="rr_sb")
    nc.vector.tensor_copy(rr_sb[:], ps_rr)
    rr2 = work.tile([128, 3 * NE], F32, tag="rr2")
    nc.scalar.square(rr2[:], ps_rr)
    rr_len2 = work.tile([128, NE], F32, tag="rr_len2")
    nc.vector.reduce_sum(
        rr_len2[:], rr2[:].rearrange("p (j c) -> p j c", c=3),
        axis=mybir.AxisListType.X,
    )
    rr_len = work.tile([128, NE], F32, tag="rr_len")
    nc.scalar.sqrt(rr_len[:], rr_len2[:])  # diagonal is exactly 0

    if stage == "geom":
        dbg_out(e_col[0:1, :])
        for p in reversed(list(ctx_pools.values())):
            p.release()
        return

    # ---------------- pT2_0: p_v0 in transposed-doubled layout ----------------
    # pT2_0[g, j*64+il] = p_v0[il, j, g] (u half, partitions 0..3)
    # pT2_0[64+g, ...] = p_v0[64+il, j, g] (d half, partitions 64..67)
    pT2_0 = big.tile([128, 8192], FP16, tag="pT2_0")
    for g in range(4):
        if g < 3:
            # TensorE transpose silently no-ops the transpose for strided
            # inputs on HW (sim transposes) -- stage through a contiguous tile
            cont = pipe2.tile([128, 128], F32, tag="contg")
            nc.vector.tensor_copy(
                cont[:], rr_sb[:].rearrange("p (j c) -> p j c", c=3)[:, :, g])
            src = cont[:]
        else:
            src = rr_len[:]
        pst = ps_sm.tile([128, 128], F32, tag="small")
        nc.tensor.transpose(pst[:], src, ident[:])  # pst[j, i] = p0[i, j, g]
        pstc = pipe2.tile([128, 128], FP16, tag="p0T")
        nc.vector.tensor_copy(pstc[:], pst[:])
        du = pT2_0[g:g + 1, :].rearrange("p (j i) -> p j i", i=64)
        dd = pT2_0[64 + g:65 + g, :].rearrange("p (j i) -> p j i", i=64)
        # d-half data also at partitions 4..7 so layer-0's p-mean chunk can
        # be a single base-0 K=8 matmul (a tile_position'd matmul cannot
        # share an accumulation group with full-K ones on HW)
        dd2 = pT2_0[4 + g:5 + g, :].rearrange("p (j i) -> p j i", i=64)
        dma(du[:], pstc[:, 0:64])
        dma(dd[:], pstc[:, 64:128])
        dma(dd2[:], pstc[:, 64:128])

    # ---------------- p-layer weights (doubled to both partition halves) ----
    Wp, Wpb, Kp = [], [], [4, 64, 64]
    for l, (wn, bn) in enumerate([("W0_w", "W0_b"), ("W1_w", "W1_b"),
                                  ("W2_w", "W2_b")]):
        K = Kp[l]
        wstage = pipe2.tile([64, 64], F32, tag="wstage")
        dma(wstage[0:K, :], ins[wn][:])
        wt = const.tile([128, 64], FP16, tag=f"wp{l}")
        nc.vector.tensor_copy(wt[0:K, :], wstage[0:K, :])
        nc.vector.tensor_copy(wt[64:64 + K, :], wstage[0:K, :])
        bc = const.tile([128, 1], F32, tag=f"wpb{l}")
        dma(bc[0:64, :], ins[bn][:].rearrange("(a k) -> a k", k=1))
        dma(bc[64:128, :], ins[bn][:].rearrange("(a k) -> a k", k=1))
        Wp.append(wt)
        Wpb.append(bc)

    # ---------------- s-weight prefetch ----------------
    # All s-layer / head weights staged to SBUF now: the DMA descriptors
    # stripe across the 16 queues and land well before the s-layers start,
    # so no matmul ever waits on HBM. Emitted AFTER the pT2_0 build DMAs so
    # those small critical transfers aren't stuck behind 3.4MB of weights.
    # V0_w chunk rows: su 0:128, sd 128:256, pm 256:264 (K=8), sv 264:392.
    WT0 = wpre.tile([128, 4 * 512], F32, tag="WT0")
    dma(WT0[:, 0:512], ins["V0_w"][0:128, :])
    dma(WT0[:, 512:1024], ins["V0_w"][128:256, :])
    dma(WT0[0:8, 1024:1536], ins["V0_w"][256:264, :])
    dma(WT0[:, 1536:2048], ins["V0_w"][264:392, :])

    # su/sd/pm weight rows (0:1152) are consumed by fp16 matmuls (their
    # lhsT carries smooth MEAN signals; fp16's 2.4e-4 rel rounding is far
    # below the tanh-chain's noise floor, and fp16 runs 1 cyc/row single
    # pass vs f32's two half-speed passes). sv rows (1152:1664) stay f32:
    # they carry the raw activations that dominate the det sensitivity.
    WTbig = {}
    WTbigh = {}
    for wk in ("V1_w", "V2_w"):
        th = wpre.tile([128, 9 * 512], FP16, tag=f"WTh_{wk}")
        for c in range(9):
            wv = wstream.tile([128, 512], F32, tag="vw")
            dma(wv[:], ins[wk][c * 128:(c + 1) * 128, :])
            nc.scalar.activation(th[:, c * 512:(c + 1) * 512], wv[:],
                                 AF.Identity)
        WTbigh[wk] = th
        t = wpre.tile([128, 4 * 512], F32, tag=f"WT_{wk}")
        dma(t[:].rearrange("p (c n) -> p c n", n=512),
            ins[wk][1152:1664, :].rearrange("(c p) n -> p c n", p=128))
        WTbig[wk] = t
    # layer 4 (after_w) feeds the heads/determinant directly: keep it exact
    tf = wpre.tile([128, 13 * 512], F32, tag="WT_after_w")
    dma(tf[:].rearrange("p (c n) -> p c n", n=512),
        ins["after_w"][:].rearrange("(c p) n -> p c n", p=128))
    WTbig["after_w"] = tf

    WTvh = {}
    for wk in ("vhu_w", "vhd_w"):
        t = wpre.tile([128, 4 * 256], F32, tag=f"WT_{wk}")
        dma(t[:].rearrange("p (c n) -> p c n", n=256),
            ins[wk][:].rearrange("(c p) n -> p c n", p=128))
        WTvh[wk] = t
    WTw = {}
    for wk in ("wu_w", "wd_w"):
        t = wpre.tile([128, 2 * 64], F32, tag=f"WT_{wk}")
        dma(t[:].rearrange("p (c n) -> p c n", n=64),
            ins[wk][:].rearrange("(c p) n -> p c n", p=128))
        WTw[wk] = t

    BT = {}
    for bk, w in (("V0_b", 512), ("V1_b", 512), ("V2_b", 512),
                  ("after_b", 512), ("vhu_b", 256), ("vhd_b", 256),
                  ("wu_b", 64), ("wd_b", 64)):
        t = wpre.tile([1, w], F32, tag=f"BT_{bk}")
        dma(t[:], ins[bk][:].rearrange("(k a) -> k a", k=1))
        BT[bk] = t

    # ones64: in0 operand for the just-in-time one-hot build in the LU loop
    ones64 = const.tile([128, 64], F32, tag="ones64")
    nc.gpsimd.memset(ones64[:], 1.0)

    # istack[p, j] = 1 iff p%64 == j  (diag extraction mask for the end)
    istack = const.tile([128, 64], F32, tag="istack")
    nc.vector.tensor_copy(istack[0:64, :], ident[0:64, 0:64])
    nc.vector.tensor_copy(istack[64:128, :], ident[64:128, 64:128])

    # ---------------- p-layers ----------------
    # t_{l+1} = tanh(W_l^T applied to p_v_l); p_v residuals kept distributed.
    t_tiles = []

    def p_layer(l, rhs_list, out_tag=None):
        """rhs_list: list of (tile, K) contributions summed pre-tanh.
        Two 512-col chunks share one [128,1024] PSUM tile (2 banks) so the
        tanh runs as ONE activation per pair: the ~370ns per-op ACT
        overhead dominates the 512-element data time, so halving the op
        count saves ~9us across the three layers."""
        out_t = big.tile([128, 8192], FP16, tag=out_tag or f"t{l + 1}")
        wt, bc = Wp[l], Wpb[l]
        for c2 in range(8):
            ps = ps_big.tile([128, 1024], F32, tag="big1024")
            n = len(rhs_list)
            for half in range(2):
                c = 2 * c2 + half
                sl = slice(c * 512, (c + 1) * 512)
                pssl = slice(half * 512, (half + 1) * 512)
                for idx, (src, K) in enumerate(rhs_list):
                    # independent accumulation group per psum region; the
                    # half-0 u-area brackets the sim's per-tensor group,
                    # the other three areas skip the (bank-global) check
                    st, sp = idx == 0, idx == n - 1
                    nc.tensor.matmul(ps[0:64, pssl], wt[0:K, :],
                                     src[0:K, sl],
                                     start=st, stop=sp, tile_position=(0, 0),
                                     skip_group_check=(half == 1))
                    # skip_group_check: the sim's zero-region tracking is
                    # bank-global, but disjoint groups are sound
                    # (per-element has_written bits); verified numerically.
                    nc.tensor.matmul(ps[64:128, pssl], wt[64:64 + K, :],
                                     src[64:64 + K, sl],
                                     start=st, stop=sp,
                                     tile_position=(64, 64),
                                     skip_group_check=True)
            nc.scalar.activation(out_t[:, c2 * 1024:(c2 + 1) * 1024], ps[:],
                                 AF.Tanh, bias=bc[:])
        t_tiles.append(out_t)
        return out_t

    t1 = p_layer(0, [(pT2_0, 4)])

    # ---------------- p means (cumulative, scaled 1/64) ----------------
    # red_l[q, j] = sum_il t_l[q, j*64+il]; pmean chunks feed s-matmul lhsT.
    def p_reduce(src, tag):
        # quarter-split: each 2.15us piece starts once its quarter of the
        # tanh output lands, instead of one 8.6us op gated on the full tile
        red = work.tile([128, 128], F32, tag=tag)
        for q in range(4):
            nc.vector.reduce_sum(
                red[:, q * 32:(q + 1) * 32],
                src[:, q * 2048:(q + 1) * 2048].rearrange(
                    "p (j i) -> p j i", i=64),
                axis=mybir.AxisListType.X,
            )
        return red

    # pT2_0 rows 0-3 = u features, rows 4-7 = d (duplicated); one K=8 block
    red0 = work.tile([128, 128], F32, tag="red0")
    pm0 = work.tile([128, 128], F32, tag="pm0")
    for q in range(4):
        nc.vector.reduce_sum(
            red0[0:8, q * 32:(q + 1) * 32],
            pT2_0[0:8, q * 2048:(q + 1) * 2048].rearrange(
                "p (j i) -> p j i", i=64),
            axis=mybir.AxisListType.X,
        )
    nc.scalar.activation(pm0[0:8, :], red0[0:8, :],
                         AF.Identity, scale=1.0 / 64.0)

    def pm_part(t, tag):
        red = p_reduce(t, "red" + tag)
        pm = work.tile([128, 128], F32, tag="pm" + tag)
        nc.scalar.activation(pm[:], red[:], AF.Identity, scale=1.0 / 64.0)
        return pm

    # ---------------- s-layers ----------------
    # Emission interleaves the s-chain INTO the p-chain: s_v1 only needs
    # s_v0+pm0, s_v2 needs pm1 (t1's reduce), etc. -- so the scheduler can
    # slot s-layer PE chunks into the p-phase's PE gaps.
    def s_means_bcast(s_v, width, lname, fp16=True, fast=False):
        """Column-mean of the u/d row-halves of s_v, broadcast to [128,128]
        lhsT tiles. Returns (su_tiles, sd_tiles), one per 128-col chunk.
        fast mode (layers 2-4): ALL means land in disjoint columns of ONE
        PSUM tile borrowed from ps_big (idle after the p-layers), staged by
        a single scalar copy -- without this the means rotate through the
        2-buffer ps_sm pool shared with the transposes, threading every
        su/sd chunk pair through a ~2.5us mean->copy->broadcast chain."""
        nch = width // 128
        su, sd = [], []
        if fast:
            psm_t = ps_big.tile([128, 1024], F32, tag="big1024")
            psm_all = psm_t[:, 0:2 * nch]
            for c in range(nch):
                for half in (0, 1):
                    base = half * 64
                    idx = 2 * c + half
                    nc.tensor.matmul(
                        psm_all[:, idx:idx + 1],
                        s_v[base:base + 64, c * 128:(c + 1) * 128],
                        inv64_col[base:base + 64, :],
                        start=True, stop=True, tile_position=(base, 0),
                        skip_group_check=(idx > 0))
            mcol_all = pipe2.tile([128, 8], F32, tag="mcolall")
            nc.scalar.activation(mcol_all[:, 0:2 * nch], psm_all,
                                 AF.Identity)
            for c in range(nch):
                for half, out_list in ((0, su), (1, sd)):
                    idx = 2 * c + half
                    bt = sbcast.tile([128, 128], FP16 if fp16 else F32,
                                     tag="sbcast" + ("h" if fp16 else "f"))
                    nc.scalar.activation(bt[:], ident[:], AF.Identity,
                                         bias=mcol_all[:, idx:idx + 1],
                                         scale=0.0)
                    out_list.append(bt)
            return su, sd
        for c in range(nch):
            sl = slice(c * 128, (c + 1) * 128)
            for half, out_list in ((0, su), (1, sd)):
                base = half * 64
                psm = ps_sm.tile([128, 1], F32, tag="small")
                nc.tensor.matmul(
                    psm[:], s_v[base:base + 64, sl],
                    inv64_col[base:base + 64, :],
                    start=True, stop=True,
                    tile_position=(base, 0),
                )
                # stage the mean column through the SCALAR engine, not DVE:
                # tiny DVE copies queue behind the 8.6us p-reduces on the
                # in-order DVE and stalled the whole s-chain ~7us.
                mcol = pipe2.tile([128, 1], F32, tag="mcol")
                nc.scalar.activation(mcol[:], psm[:], AF.Identity)
                bt = sbcast.tile([128, 128], FP16 if fp16 else F32,
                                 tag="sbcast" + ("h" if fp16 else "f"))
                nc.scalar.activation(bt[:], ident[:], AF.Identity,
                                     bias=mcol[:], scale=0.0)
                out_list.append(bt)
        return su, sd

    def s_transposes(s_v, width, lname):
        out = []
        for c in range(width // 128):
            sl = slice(c * 128, (c + 1) * 128)
            pst = ps_sm.tile([128, 128], F32, tag="small")
            nc.tensor.transpose(pst[:], s_v[:, sl], ident[:])
            svt = svtp.tile([128, 128], F32, tag="svT")
            nc.scalar.activation(svt[:], pst[:], AF.Identity)
            out.append(svt)
        return out

    def s_layer(lname, chunks, bias_tile):
        """chunks: (lhsT_ap, w_ap) pairs accumulated into one PSUM group.
        All matmuls are plain f32: f32r's truncation noise gets
        chaos-amplified through the 4-layer chain and the ill-conditioned
        logdet (measured ~100 absolute shift on HW); exact f32 at 4
        cycles/row is the price of correctness. Returns s_v [128,512] f32."""
        ps_t = ps_s.tile([128, 512], F32, tag="sx512")
        ps = ps_t[:]
        for idx, (lhsT, wap) in enumerate(chunks):
            nc.tensor.matmul(ps, lhsT, wap, start=(idx == 0), stop=False)
        nc.tensor.matmul(ps, ones_row[:], bias_tile[:],
                         start=False, stop=True)
        s_v = work.tile([128, 512], F32, tag=f"sv{lname}")
        nc.scalar.activation(s_v[:], ps, AF.Tanh)
        return s_v

    # layer 0: fin = 392 = su(128) sd(128) pu+pd(8) sv(128)
    sv0T = s_transposes(s_v0, 128, "0")
    su0, sd0 = s_means_bcast(s_v0, 128, "0", fp16=False)
    s_v1 = s_layer(
        "1",
        [(sv0T[0][:], WT0[:, 1536:2048]),
         (su0[0][:], WT0[:, 0:512]), (sd0[0][:], WT0[:, 512:1024]),
         (pm0[0:8, :], WT0[0:8, 1024:1536])],
        BT["V0_b"],
    )

    # layers 1, 2, after: fin = 1664 = su(512) sd(512) pu+pd(128) sv(512)
    # Vw chunk c occupies WT[:, c*512:(c+1)*512]; rows: su c0-3, sd c4-7,
    # pm c8, sv c9-12.
    def big_s_layer(lname, wth, wtf, bias_tile, s_v, pm_cum):
        svT = s_transposes(s_v, 512, lname)
        su, sd = s_means_bcast(s_v, 512, lname, fp16=(wth is not None),
                                fast=True)
        if wth is not None:
            # pm_cum enters its fp16 chunk rounded once per layer
            pmh = work.tile([128, 128], FP16, tag=f"pmh{lname}")
            nc.scalar.activation(pmh[:], pm_cum[:], AF.Identity)

        # chunk ORDER within the PSUM accumulation group is free; put the
        # transpose + pm chunks (ready ~1us after the tanh) first so the
        # means' psm->mcol->broadcast latency hides behind them.
        # pm LAST: its reduce is the slowest input (gated on the full
        # previous p-layer); everything else is ready within ~1us.
        chunks = []
        if wth is not None:
            for c in range(4):
                chunks.append((svT[c][:], wtf[:, c * 512:(c + 1) * 512]))
            for c in range(4):
                chunks.append((su[c][:], wth[:, c * 512:(c + 1) * 512]))
            for c in range(4):
                chunks.append((sd[c][:], wth[:, (4 + c) * 512:(5 + c) * 512]))
            # pu rows 1024:1088 / pd 1088:1152 are contiguous in Vw; pm_cum
            # holds pu at partitions 0:64, pd at 64:128 -- one K=128 chunk.
            chunks.append((pmh[:], wth[:, 8 * 512:9 * 512]))
        else:
            for c in range(4):
                chunks.append((svT[c][:], wtf[:, (9 + c) * 512:(10 + c) * 512]))
            for c in range(4):
                chunks.append((su[c][:], wtf[:, c * 512:(c + 1) * 512]))
            for c in range(4):
                chunks.append((sd[c][:], wtf[:, (4 + c) * 512:(5 + c) * 512]))
            chunks.append((pm_cum[:], wtf[:, 8 * 512:9 * 512]))
        return s_layer(lname, chunks, bias_tile)

    if stage == "s1" and dbg_d is not None:
        sv1f = work.tile([128, 512], F32, tag="sv1f")
        nc.scalar.activation(sv1f[:], s_v1[:], AF.Identity)
        dma(dbg_d[:], sv1f[:])
        dbg_out(s_v1[0:1, 0:1])
        for p in reversed(list(ctx_pools.values())):
            p.release()
        return

    t2 = p_layer(1, [(t1, 64)])
    pm1 = pm_part(t1, "1")
    s_v2 = big_s_layer("2", WTbigh["V1_w"], WTbig["V1_w"], BT["V1_b"], s_v1, pm1)
    # residual p2 = t1 + t2 pre-summed on the DVE (16 fp16 adds, mid-phase
    # slack) instead of doubling layer-3's matmul count (32 extra K=64
    # matmuls on the jointly-bound PE). Same arithmetic, re-associated.
    p2 = big.tile([128, 8192], FP16, tag="p2")
    for c in range(16):
        sl = slice(c * 512, (c + 1) * 512)
        nc.vector.tensor_tensor(p2[:, sl], t1[:, sl], t2[:, sl], op=Alu.add)
    # t3 reuses pT2_0's SBUF slot (pT2_0 is dead after layer 0 + its reduce)
    t3 = p_layer(2, [(p2, 64)], out_tag="pT2_0")
    pm2 = pm_part(t2, "2")
    # accumulate means in-place: pm1 += pm2 (after layer 2 consumed pm1)
    nc.vector.tensor_tensor(pm1[:], pm1[:], pm2[:], op=Alu.add)
    s_v3 = big_s_layer("3", WTbigh["V2_w"], WTbig["V2_w"], BT["V2_b"], s_v2, pm1)
    pm3 = pm_part(t3, "3")
    nc.vector.tensor_tensor(pm1[:], pm1[:], pm3[:], op=Alu.add)
    s_v4 = big_s_layer("4", None, WTbig["after_w"], BT["after_b"], s_v3, pm1)

    if stage == "s":
        dbg_out(s_v4[0:1, 0:1])
        for p in reversed(list(ctx_pools.values())):
            p.release()
        return

    # ---------------- heads ----------------
    sv4T = s_transposes(s_v4, 512, "4")

    def head_half(wkey, bkey):
        ps = ps_sm.tile([64, 256], F32, tag="small")
        base = 0 if wkey == "vhu_w" else 64
        wt = WTvh[wkey]
        for c in range(4):
            nc.tensor.matmul(ps[:], sv4T[c][:, base:base + 64],
                             wt[:, c * 256:(c + 1) * 256],
                             start=(c == 0), stop=False)
        nc.tensor.matmul(ps[:], ones_row[:, 0:64], BT[bkey][:],
                         start=False, stop=True)
        sh = work.tile([64, 256], F32, tag="sh" + wkey)
        nc.vector.tensor_copy(sh[:], ps[:])
        return sh

    shu = head_half("vhu_w", "vhu_b")
    shd = head_half("vhd_w", "vhd_b")

    def head_T(sh, nm):
        out = []
        for c in range(2):
            pst = ps_sm.tile([128, 128], F32, tag="small")
            nc.tensor.transpose(pst[0:128, 0:64],
                                sh[:, c * 128:(c + 1) * 128],
                                ident[0:64, 0:64])
            ht = svtp.tile([128, 64], F32, tag="ht")
            nc.vector.tensor_copy(ht[:], pst[0:128, 0:64])
            out.append(ht)
        return out

    shuT = head_T(shu, "u")
    shdT = head_T(shd, "d")

    ps_A = ps_sm.tile([128, 64], F32, tag="small")
    for c in range(2):
        nc.tensor.matmul(ps_A[0:64, :], shuT[c][:],
                         WTw["wu_w"][:, c * 64:(c + 1) * 64],
                         start=(c == 0), stop=False, tile_position=(0, 0))
    nc.tensor.matmul(ps_A[0:64, :], ones_row[:, 0:64], BT["wu_b"][:],
                     start=False, stop=True, tile_position=(0, 0))
    for c in range(2):
        nc.tensor.matmul(ps_A[64:128, :], shdT[c][:],
                         WTw["wd_w"][:, c * 64:(c + 1) * 64],
                         start=(c == 0), stop=False, tile_position=(0, 64))
    nc.tensor.matmul(ps_A[64:128, :], ones_row[:, 0:64], BT["wd_b"][:],
                     start=False, stop=True, tile_position=(0, 64))

    # orb = s_w * (sum_j exp(-|ra|)) row-scale; stacked [A_u; A_d]
    A_sb = work.tile([128, 64], F32, tag="A_sb")
    nc.vector.tensor_scalar_mul(A_sb[:], ps_A[:], e_col[:])

    if stage == "heads":
        dbg_out(A_sb[0:1, 0:1])
        for p in reversed(list(ctx_pools.values())):
            p.release()
        return

    # warm the Abs/Ln activation tables now: the scalar engine is idle for
    # the whole ~90us LU phase, so the two ~1.3us ACT_TABLE_LOADs happen in
    # its shadow instead of on the logdet tail's critical path. Reading
    # A_sb (not a const) pins these AFTER the heads: the scheduler would
    # otherwise hoist them to t~8us where the p/s-phase Tanh/Identity/Exp
    # loads evict the tables again.
    warm = const.tile([1, 1], F32, tag="warm")
    nc.scalar.activation(warm[:], A_sb[0:1, 0:1], AF.Abs)
    nc.scalar.activation(warm[:], warm[:], AF.Ln)

    # ---------------- stacked unpivoted LU ----------------
    # Per step ONE matmul broadcasts the pivot row of each half to all
    # partitions of that half, straight into PSUM; the rhs is narrowed to
    # A[:, k:] (only live columns). The one-hot lhsT Cb (Cb[i,p] = 1 iff
    # i == k+64*(p>=64)) is built JUST-IN-TIME one step ahead with two DVE
    # tensor_scalar ops (ones64 * ident column) into a double-buffered
    # tile: those fill DVE's idle window while it waits for the matmul, so
    # they cost nothing. Then on DVE:
    #   r  = 1/ub[:,0]                      (per-partition u_kk broadcast)
    #   m  = (A[:,k] * r) * negmask[:,k]    (masked negated multipliers)
    #   A[:, k+1:] = (ub[:,1:] * m) + A     (fused rank-1 update, one op)
    # Rows <= k keep exact-zero multipliers so finished U rows are never
    # corrupted; diag(U) is read off the final A afterwards.
    def build_cb(k):
        # built on the SCALAR engine (idle during the LU) via the
        # bias-broadcast trick, so the DVE chain never sees these ops
        cb = lu.tile([128, 128], F32, tag=f"cb{k % 2}")
        nc.scalar.activation(cb[:, 0:64], ones64[:], AF.Identity,
                             bias=ident[:, k:k + 1], scale=0.0)
        nc.scalar.activation(cb[:, 64:128], ones64[:], AF.Identity,
                             bias=ident[:, 64 + k:65 + k], scale=0.0)
        return cb

    n_lu = {"lu16": 16}.get(stage, 63)
    cb = build_cb(0)
    for k in range(n_lu):
        w = 64 - k
        ps_ub = ps_lu.tile([128, 64], F32, tag="ub")
        # NOTE f32r here fails BIR verification: every producer writing A_sb
        # would have to round its output to f32r (changing LU numerics), so
        # the broadcast matmul stays plain f32 (2-pass at M=128, ~640ns).
        nc.tensor.matmul(ps_ub[:, 0:w], cb[:], A_sb[:, k:64],
                         start=True, stop=True)
        if k + 1 < n_lu:
            cb = build_cb(k + 1)
        rcol = lu.tile([128, 1], F32, tag="rcol")
        nc.vector.reciprocal(rcol[:], ps_ub[:, 0:1])
        m = lu.tile([128, 1], F32, tag="m")
        nc.vector.scalar_tensor_tensor(m[:], A_sb[:, k:k + 1], rcol[:],
                                       negmask[:, k:k + 1],
                                       op0=Alu.mult, op1=Alu.mult)
        nc.vector.scalar_tensor_tensor(
            A_sb[:, k + 1:64], ps_ub[:, 1:w], m[:], A_sb[:, k + 1:64],
            op0=Alu.mult, op1=Alu.add)

    if stage == "lu16":
        dbg_out(A_sb[0:1, 0:1])
        for p in reversed(list(ctx_pools.values())):
            p.release()
        return

    # logdet = sum_p ln|diag| ; diag[p] = A[p, p%64] via masked reduce
    prod = work.tile([128, 64], F32, tag="prod")
    dcol = work.tile([128, 1], F32, tag="dcol")
    nc.vector.tensor_tensor(prod[:], A_sb[:], istack[:], op=Alu.mult)
    nc.vector.reduce_sum(dcol[:], prod[:], axis=mybir.AxisListType.X)
    dabs = work.tile([128, 1], F32, tag="dabs")
    nc.scalar.activation(dabs[:], dcol[:], AF.Abs)
    dln = work.tile([128, 1], F32, tag="dln")
    nc.scalar.activation(dln[:], dabs[:], AF.Ln)
    # cross-partition sum: transpose the column to a row, reduce along free
    ps_out = ps_sm.tile([1, 128], F32, tag="small")
    nc.tensor.transpose(ps_out[:], dln[:], ident[:])
    s_row = work.tile([1, 128], F32, tag="s_row")
    nc.vector.tensor_copy(s_row[:], ps_out[:])
    tot = work.tile([1, 1], F32, tag="tot")
    nc.vector.reduce_sum(tot[:], s_row[:], axis=mybir.AxisListType.X)
    dma(out_d[:], tot[:])

    for p in reversed(list(ctx_pools.values())):
        p.release()


_NC_CACHE = {}


def build_nc():
    if "nc" in _NC_CACHE:
        return _NC_CACHE["nc"]
    import os
    nc = bacc.Bacc("TRN2", target_bir_lowering=False, debug=False)
    ins = {}
    for name, shape in INPUT_SPECS:
        ins[name] = nc.dram_tensor(name, list(shape), F32,
                                   kind="ExternalInput").ap()
    out_d = nc.dram_tensor("out", [1, 1], F32, kind="ExternalOutput").ap()
    dbg_d = None
    if os.environ.get("KSTAGE", "full") in ("s1", "pm0"):
        dbg_d = nc.dram_tensor("dbgout", [128, 512], F32,
                               kind="ExternalOutput").ap()
    with tile.TileContext(nc) as tc:
        _program(tc, nc, ins, out_d, dbg_d)
    nc.compile()
    _NC_CACHE["nc"] = nc
    return nc


def kernel(**inputs) -> np.ndarray:
    from concourse.bass_utils import run_bass_kernel_spmd

    nc = build_nc()
    in_map = {name: np.ascontiguousarray(np.asarray(inputs[name],
                                                    dtype=np.float32))
              for name, _ in INPUT_SPECS}
    in_maps = [in_map for _ in range(8)]
    res = run_bass_kernel_spmd(nc, in_maps, core_ids=list(range(8)))
    out = res.results[0]["out"]
    return np.float32(out.reshape(())[()])


# revision 61
# speedup vs baseline: 1.0002x; 1.0002x over previous
"""Trainium2 Bass kernel for the FermiNet-style single-configuration ansatz.

Computes log|psi| = logdet(orb_u) + logdet(orb_d) for one electron
configuration. The whole forward runs replicated on 8 NeuronCores (the
problem is tiny; inter-core collectives have a ~7-20us latency floor that
dwarfs the ~1 GFLOP of compute, so replication is the fastest correct
distribution) and core 0's scalar output is returned.

Layout choices (see inline comments):
  - p-tensor kept transposed+doubled: pT2[q, j*64 + i_local], q<64 = feature g
    for spin-up electrons (i<64), q>=64 = feature g for spin-down. This makes
    the per-pair feature matmul a K=64 contraction over partitions, lets
    spin-up/down run concurrently in separate PE array quadrants
    (tile_position), and makes the i-mean a free-dim segmented reduce.
  - residuals p2 = t2 + t1 are never materialized; the matmul and the means
    distribute over the sum (tanh outputs t_l are kept separately).
  - p-mean contributions to the s-layers use ONE cumulative [128,128] tile
    (DVE adds between layers) instead of repeated Vw chunks.
  - s-layer weights are all prefetched to SBUF right after the p-tensor
    build (descriptor-striped big DMAs), so s-layer matmuls never stall.
  - biases along the free dim are added as rank-1 (ones x b) matmuls into the
    same PSUM accumulation group.
  - logdet via unpivoted rank-1 Gaussian elimination on the stacked [A_u;A_d]
    [128,64] tile. Per step ONE K=128 one-hot matmul broadcasts the pivot
    row of both halves straight to PSUM, then 3 DVE ops (reciprocal /
    multiplier / fused scalar_tensor_tensor rank-1 update). The one-hot
    lhsT is built just-in-time one step ahead on the otherwise-idle SCALAR
    engine (bias-broadcast trick), so neither PE nor DVE pays for it.
    Unpivoted LU is stable here (growth factor ~700, logdet error ~6e-3 in
    f32 vs 2e-2 rtol). diag(U) is read off the final A at the end.

Hard-won platform notes (cost ~6 HW iterations to learn):
  - matmul PE time scales with OUT FREE SIZE (N) x cycles/row(rhs dtype),
    NOT with K; f32 runs as 2 half-speed passes when M > 64.
  - bulk GPSIMD ucode is poison: consumers of ANY gpsimd output wait for
    the engine's full queue DRAIN (a 16us CC build stalled all DMAs 15us).
  - stride-0 (broadcast_to) DMA APs fail neuronxcc codegen.
  - f32r matmuls require every PRODUCER of their operands to round to
    f32r (BIR verifier), and still hit an ISA check failure here.
  - tensor_tensor_reduce crashes the exec unit on TRN2 HW (sim-only op).
  - act tables: ~6-7 live; warm Abs/Ln AFTER the last Tanh/Identity use
    (pin the warm-up late via a data dependency or the scheduler hoists it).
"""

import numpy as np

import concourse.bass as bass
import concourse.bacc as bacc
import concourse.mybir as mybir
import concourse.tile as tile
from concourse import bass_isa, masks

F32 = mybir.dt.float32
F32R = mybir.dt.float32r
FP16 = mybir.dt.float16
AF = mybir.ActivationFunctionType
Alu = mybir.AluOpType

NE, NA, NSV, NPV, NU = 128, 32, 512, 64, 64

INPUT_SPECS = [
    ("r", (128, 3)), ("a", (32, 3)),
    ("V0_w", (392, 512)), ("V0_b", (512,)),
    ("V1_w", (1664, 512)), ("V1_b", (512,)),
    ("V2_w", (1664, 512)), ("V2_b", (512,)),
    ("W0_w", (4, 64)), ("W0_b", (64,)),
    ("W1_w", (64, 64)), ("W1_b", (64,)),
    ("W2_w", (64, 64)), ("W2_b", (64,)),
    ("after_w", (1664, 512)), ("after_b", (512,)),
    ("vhu_w", (512, 256)), ("vhu_b", (256,)),
    ("vhd_w", (512, 256)), ("vhd_b", (256,)),
    ("wu_w", (256, 64)), ("wu_b", (64,)),
    ("wd_w", (256, 64)), ("wd_b", (64,)),
]


def _r(ap):
    return ap.bitcast(F32R)


def _program(tc, nc, ins, out_d, dbg_d=None):
    import os
    stage = os.environ.get("KSTAGE", "full")
    ctx_pools = {}

    def pool(name, bufs, space="SBUF"):
        if name not in ctx_pools:
            ctx_pools[name] = tc.alloc_tile_pool(name=name, bufs=bufs,
                                                 space=space)
        return ctx_pools[name]

    const = pool("const", 1)
    work = pool("work", 1)
    pipe2 = pool("pipe2", 2)
    sbcast = pool("sbcast", 8)
    svtp = pool("svtp", 4)
    big = pool("big", 1)
    wpre = pool("wpre", 1)
    wstream = pool("wstream", 4)
    lu = pool("lu", 3)
    ps_big = pool("ps_big", 2, space="PSUM")
    ps_sm = pool("ps_sm", 2, space="PSUM")
    ps_lu = pool("ps_lu", 1, space="PSUM")
    ps_s = pool("ps_sx", 1, space="PSUM")

    dma = nc.sync.dma_start

    # ---------------- constants ----------------
    ident = const.tile([128, 128], F32, tag="ident")
    masks.make_identity(nc, ident[:])
    ones_row = const.tile([1, 128], F32, tag="ones_row")
    nc.gpsimd.memset(ones_row[:], 1.0)
    inv64_col = const.tile([128, 1], F32, tag="inv64")
    nc.gpsimd.memset(inv64_col[:], 1.0 / 64.0)

    # LU strict-lower mask (negated): negmask[p, k] = -1 iff (p % 64) > k.
    # affine_select indexes partitions view-relative (probed in sim), so the
    # same base works for both halves.
    negmask = const.tile([128, 64], F32, tag="negmask")
    nc.gpsimd.memset(negmask[:], -1.0)
    for half in range(2):
        nc.gpsimd.affine_select(
            out=negmask[half * 64:(half + 1) * 64, :],
            in_=negmask[half * 64:(half + 1) * 64, :],
            pattern=[[-1, 64]], compare_op=Alu.is_ge,
            fill=0.0, base=-1, channel_multiplier=1)

    # ---------------- geometry ----------------
    r_sb = work.tile([128, 3], F32, tag="r_sb")
    dma(r_sb[:], ins["r"][:])

    # rT4 = [r^T ; ones] as [4, 128]
    psr = ps_sm.tile([4, 128], F32, tag="small")
    nc.tensor.transpose(psr[0:3, :], r_sb[:], ident[:])
    rT4 = const.tile([4, 128], F32, tag="rT4")
    nc.gpsimd.memset(rT4[:], 1.0)  # row 3 stays ones
    nc.vector.tensor_copy(rT4[0:3, :], psr[0:3, :])

    def delta_rows(t, nj, val):
        """t[c, j*3+k] = val*(k == c) for c in 0..2; row 3 zeroed (DMA after).
        Compute-engine APs must start at partition 0/32/64/96, so build the
        delta pattern with one affine_select over all 4 rows. NOTE: keep
        total GPSIMD ucode work tiny -- every consumer of ANY gpsimd output
        waits for the engine's full queue drain."""
        nc.gpsimd.memset(t[:], val)
        nc.gpsimd.affine_select(
            out=t[:], in_=t[:], pattern=[[0, nj], [1, 3]],
            compare_op=Alu.is_equal, fill=0.0, base=0, channel_multiplier=-1)

    def into_row3(t, src_flat, width, tag, scale):
        st = pipe2.tile([1, width], F32, tag=tag)
        dma(st[:], src_flat)
        nc.scalar.mul(st[:], st[:], scale)
        dma(t[3:4, :], st[:])

    # Wra[4, 96]: ra = [r|1] @ Wra,  ra[i, j*3+c] = r[i,c] - a[j,c]
    # ra[i, j] = r[i] - a[j]
    Wra = const.tile([4, 3 * NA], F32, tag="Wra")
    delta_rows(Wra, NA, 1.0)
    into_row3(Wra, ins["a"][:].rearrange("a b -> (a b)"), 3 * NA, "nga", -1.0)

    ps_ra_t = ps_big.tile([128, 1024], F32, tag="big1024")
    ps_ra = ps_ra_t[:, 0:3 * NA]
    nc.tensor.matmul(ps_ra, rT4[:], Wra[:], start=True, stop=True)
    ra_sb = work.tile([128, 3 * NA], F32, tag="ra_sb")
    nc.vector.tensor_copy(ra_sb[:], ps_ra)
    ra2 = work.tile([128, 3 * NA], F32, tag="ra2")
    nc.scalar.square(ra2[:], ps_ra)
    ra_len2 = work.tile([128, NA], F32, tag="ra_len2")
    nc.vector.reduce_sum(
        ra_len2[:], ra2[:].rearrange("p (j c) -> p j c", c=3),
        axis=mybir.AxisListType.X,
    )
    ra_len = work.tile([128, NA], F32, tag="ra_len")
    nc.scalar.sqrt(ra_len[:], ra_len2[:])
    # e_col[i] = sum_j exp(-|r_i - a_j|)
    e_col = const.tile([128, 1], F32, tag="e_col")
    eexp = work.tile([128, NA], F32, tag="eexp")
    nc.scalar.activation(eexp[:], ra_len[:], AF.Exp, scale=-1.0,
                         accum_out=e_col[:])

    def dbg_out(src_ap):
        o = work.tile([1, 1], F32, tag="out_sb")
        nc.scalar.mul(o[:], src_ap, 1.0)
        dma(out_d[:], o[:])

    # s_v0 [128, 128]: interleaved [ra_x, ra_y, ra_z, |ra|] per atom
    s_v0 = work.tile([128, 128], F32, tag="s_v0")
    v4 = s_v0[:].rearrange("p (j k) -> p j k", k=4)
    nc.scalar.activation(v4[:, :, 0:3],
                         ra_sb[:].rearrange("p (j c) -> p j c", c=3),
                         AF.Identity)
    nc.scalar.activation(v4[:, :, 3:4],
                         ra_len[:].rearrange("p (j k) -> p j k", k=1),
                         AF.Identity)

    # rr: Wrr[4, 384], rr = [r|1] @ Wrr, rr[i, j*3+c] = r[i,c] - r[j,c]
    # reference convention: rr[i, j] = r[j] - r[i]
    Wrr = const.tile([4, 3 * NE], F32, tag="Wrr")
    delta_rows(Wrr, NE, -1.0)
    into_row3(Wrr, ins["r"][:].rearrange("a b -> (a b)"), 3 * NE, "ngr", 1.0)

    ps_rr_t = ps_big.tile([128, 1024], F32, tag="big1024")
    ps_rr = ps_rr_t[:, 0:3 * NE]
    nc.tensor.matmul(ps_rr, rT4[:], Wrr[:], start=True, stop=True)
    rr_sb = work.tile([128, 3 * NE], F32, tag="rr_sb")
    nc.vector.tensor_copy(rr_sb[:], ps_rr)
    rr2 = work.tile([128, 3 * NE], F32, tag="rr2")
    nc.scalar.square(rr2[:], ps_rr)
    rr_len2 = work.tile([128, NE], F32, tag="rr_len2")
    nc.vector.reduce_sum(
        rr_len2[:], rr2[:].rearrange("p (j c) -> p j c", c=3),
        axis=mybir.AxisListType.X,
    )
    rr_len = work.tile([128, NE], F32, tag="rr_len")
    nc.scalar.sqrt(rr_len[:], rr_len2[:])  # diagonal is exactly 0

    if stage == "geom":
        dbg_out(e_col[0:1, :])
        for p in reversed(list(ctx_pools.values())):
            p.release()
        return

    # ---------------- pT2_0: p_v0 in transposed-doubled layout ----------------
    # pT2_0[g, j*64+il] = p_v0[il, j, g] (u half, partitions 0..3)
    # pT2_0[64+g, ...] = p_v0[64+il, j, g] (d half, partitions 64..67)
    pT2_0 = big.tile([128, 8192], FP16, tag="pT2_0")
    for g in range(4):
        if g < 3:
            # TensorE transpose silently no-ops the transpose for strided
            # inputs on HW (sim transposes) -- stage through a contiguous tile
            cont = pipe2.tile([128, 128], F32, tag="contg")
            nc.vector.tensor_copy(
                cont[:], rr_sb[:].rearrange("p (j c) -> p j c", c=3)[:, :, g])
            src = cont[:]
        else:
            src = rr_len[:]
        pst = ps_sm.tile([128, 128], F32, tag="small")
        nc.tensor.transpose(pst[:], src, ident[:])  # pst[j, i] = p0[i, j, g]
        pstc = pipe2.tile([128, 128], FP16, tag="p0T")
        nc.vector.tensor_copy(pstc[:], pst[:])
        du = pT2_0[g:g + 1, :].rearrange("p (j i) -> p j i", i=64)
        dd = pT2_0[64 + g:65 + g, :].rearrange("p (j i) -> p j i", i=64)
        # d-half data also at partitions 4..7 so layer-0's p-mean chunk can
        # be a single base-0 K=8 matmul (a tile_position'd matmul cannot
        # share an accumulation group with full-K ones on HW)
        dd2 = pT2_0[4 + g:5 + g, :].rearrange("p (j i) -> p j i", i=64)
        dma(du[:], pstc[:, 0:64])
        dma(dd[:], pstc[:, 64:128])
        dma(dd2[:], pstc[:, 64:128])

    # ---------------- p-layer weights (doubled to both partition halves) ----
    Wp, Wpb, Kp = [], [], [4, 64, 64]
    for l, (wn, bn) in enumerate([("W0_w", "W0_b"), ("W1_w", "W1_b"),
                                  ("W2_w", "W2_b")]):
        K = Kp[l]
        wstage = pipe2.tile([64, 64], F32, tag="wstage")
        dma(wstage[0:K, :], ins[wn][:])
        wt = const.tile([128, 64], FP16, tag=f"wp{l}")
        nc.vector.tensor_copy(wt[0:K, :], wstage[0:K, :])
        nc.vector.tensor_copy(wt[64:64 + K, :], wstage[0:K, :])
        bc = const.tile([128, 1], F32, tag=f"wpb{l}")
        dma(bc[0:64, :], ins[bn][:].rearrange("(a k) -> a k", k=1))
        dma(bc[64:128, :], ins[bn][:].rearrange("(a k) -> a k", k=1))
        Wp.append(wt)
        Wpb.append(bc)

    # ---------------- s-weight prefetch ----------------
    # All s-layer / head weights staged to SBUF now: the DMA descriptors
    # stripe across the 16 queues and land well before the s-layers start,
    # so no matmul ever waits on HBM. Emitted AFTER the pT2_0 build DMAs so
    # those small critical transfers aren't stuck behind 3.4MB of weights.
    # V0_w chunk rows: su 0:128, sd 128:256, pm 256:264 (K=8), sv 264:392.
    WT0 = wpre.tile([128, 4 * 512], F32, tag="WT0")
    dma(WT0[:, 0:512], ins["V0_w"][0:128, :])
    dma(WT0[:, 512:1024], ins["V0_w"][128:256, :])
    dma(WT0[0:8, 1024:1536], ins["V0_w"][256:264, :])
    dma(WT0[:, 1536:2048], ins["V0_w"][264:392, :])

    # su/sd/pm weight rows (0:1152) are consumed by fp16 matmuls (their
    # lhsT carries smooth MEAN signals; fp16's 2.4e-4 rel rounding is far
    # below the tanh-chain's noise floor, and fp16 runs 1 cyc/row single
    # pass vs f32's two half-speed passes). sv rows (1152:1664) stay f32:
    # they carry the raw activations that dominate the det sensitivity.
    WTbig = {}
    WTbigh = {}
    for wk in ("V1_w", "V2_w"):
        th = wpre.tile([128, 9 * 512], FP16, tag=f"WTh_{wk}")
        for c in range(9):
            wv = wstream.tile([128, 512], F32, tag="vw")
            dma(wv[:], ins[wk][c * 128:(c + 1) * 128, :])
            nc.scalar.activation(th[:, c * 512:(c + 1) * 512], wv[:],
                                 AF.Identity)
        WTbigh[wk] = th
        t = wpre.tile([128, 4 * 512], F32, tag=f"WT_{wk}")
        dma(t[:].rearrange("p (c n) -> p c n", n=512),
            ins[wk][1152:1664, :].rearrange("(c p) n -> p c n", p=128))
        WTbig[wk] = t
    # layer 4 (after_w) feeds the heads/determinant directly: keep it exact
    tf = wpre.tile([128, 13 * 512], F32, tag="WT_after_w")
    dma(tf[:].rearrange("p (c n) -> p c n", n=512),
        ins["after_w"][:].rearrange("(c p) n -> p c n", p=128))
    WTbig["after_w"] = tf

    WTvh = {}
    for wk in ("vhu_w", "vhd_w"):
        t = wpre.tile([128, 4 * 256], F32, tag=f"WT_{wk}")
        dma(t[:].rearrange("p (c n) -> p c n", n=256),
            ins[wk][:].rearrange("(c p) n -> p c n", p=128))
        WTvh[wk] = t
    WTw = {}
    for wk in ("wu_w", "wd_w"):
        t = wpre.tile([128, 2 * 64], F32, tag=f"WT_{wk}")
        dma(t[:].rearrange("p (c n) -> p c n", n=64),
            ins[wk][:].rearrange("(c p) n -> p c n", p=128))
        WTw[wk] = t

    BT = {}
    for bk, w in (("V0_b", 512), ("V1_b", 512), ("V2_b", 512),
                  ("after_b", 512), ("vhu_b", 256), ("vhd_b", 256),
                  ("wu_b", 64), ("wd_b", 64)):
        t = wpre.tile([1, w], F32, tag=f"BT_{bk}")
        dma(t[:], ins[bk][:].rearrange("(k a) -> k a", k=1))
        BT[bk] = t

    # ones64: in0 operand for the just-in-time one-hot build in the LU loop
    ones64 = const.tile([128, 64], F32, tag="ones64")
    nc.gpsimd.memset(ones64[:], 1.0)

    # istack[p, j] = 1 iff p%64 == j  (diag extraction mask for the end)
    istack = const.tile([128, 64], F32, tag="istack")
    nc.vector.tensor_copy(istack[0:64, :], ident[0:64, 0:64])
    nc.vector.tensor_copy(istack[64:128, :], ident[64:128, 64:128])

    # ---------------- p-layers ----------------
    # t_{l+1} = tanh(W_l^T applied to p_v_l); p_v residuals kept distributed.
    t_tiles = []

    def p_layer(l, rhs_list, out_tag=None):
        """rhs_list: list of (tile, K) contributions summed pre-tanh.
        Two 512-col chunks share one [128,1024] PSUM tile (2 banks) so the
        tanh runs as ONE activation per pair: the ~370ns per-op ACT
        overhead dominates the 512-element data time, so halving the op
        count saves ~9us across the three layers."""
        out_t = big.tile([128, 8192], FP16, tag=out_tag or f"t{l + 1}")
        wt, bc = Wp[l], Wpb[l]
        for c2 in range(8):
            ps = ps_big.tile([128, 1024], F32, tag="big1024")
            n = len(rhs_list)
            for half in range(2):
                c = 2 * c2 + half
                sl = slice(c * 512, (c + 1) * 512)
                pssl = slice(half * 512, (half + 1) * 512)
                for idx, (src, K) in enumerate(rhs_list):
                    # independent accumulation group per psum region; the
                    # half-0 u-area brackets the sim's per-tensor group,
                    # the other three areas skip the (bank-global) check
                    st, sp = idx == 0, idx == n - 1
                    nc.tensor.matmul(ps[0:64, pssl], wt[0:K, :],
                                     src[0:K, sl],
                                     start=st, stop=sp, tile_position=(0, 0),
                                     skip_group_check=(half == 1))
                    # skip_group_check: the sim's zero-region tracking is
                    # bank-global, but disjoint groups are sound
                    # (per-element has_written bits); verified numerically.
                    nc.tensor.matmul(ps[64:128, pssl], wt[64:64 + K, :],
                                     src[64:64 + K, sl],
                                     start=st, stop=sp,
                                     tile_position=(64, 64),
                                     skip_group_check=True)
            nc.scalar.activation(out_t[:, c2 * 1024:(c2 + 1) * 1024], ps[:],
                                 AF.Tanh, bias=bc[:])
        t_tiles.append(out_t)
        return out_t

    t1 = p_layer(0, [(pT2_0, 4)])

    # ---------------- p means (cumulative, scaled 1/64) ----------------
    # red_l[q, j] = sum_il t_l[q, j*64+il]; pmean chunks feed s-matmul lhsT.
    def p_reduce(src, tag):
        # quarter-split: each 2.15us piece starts once its quarter of the
        # tanh output lands, instead of one 8.6us op gated on the full tile
        red = work.tile([128, 128], F32, tag=tag)
        for q in range(4):
            nc.vector.reduce_sum(
                red[:, q * 32:(q + 1) * 32],
                src[:, q * 2048:(q + 1) * 2048].rearrange(
                    "p (j i) -> p j i", i=64),
                axis=mybir.AxisListType.X,
            )
        return red

    # pT2_0 rows 0-3 = u features, rows 4-7 = d (duplicated); one K=8 block
    red0 = work.tile([128, 128], F32, tag="red0")
    pm0 = work.tile([128, 128], F32, tag="pm0")
    for q in range(4):
        nc.vector.reduce_sum(
            red0[0:8, q * 32:(q + 1) * 32],
            pT2_0[0:8, q * 2048:(q + 1) * 2048].rearrange(
                "p (j i) -> p j i", i=64),
            axis=mybir.AxisListType.X,
        )
    nc.scalar.activation(pm0[0:8, :], red0[0:8, :],
                         AF.Identity, scale=1.0 / 64.0)

    def pm_part(t, tag):
        red = p_reduce(t, "red" + tag)
        pm = work.tile([128, 128], F32, tag="pm" + tag)
        nc.scalar.activation(pm[:], red[:], AF.Identity, scale=1.0 / 64.0)
        return pm

    # ---------------- s-layers ----------------
    # Emission interleaves the s-chain INTO the p-chain: s_v1 only needs
    # s_v0+pm0, s_v2 needs pm1 (t1's reduce), etc. -- so the scheduler can
    # slot s-layer PE chunks into the p-phase's PE gaps.
    def s_means_bcast(s_v, width, lname, fp16=True, fast=False):
        """Column-mean of the u/d row-halves of s_v, broadcast to [128,128]
        lhsT tiles. Returns (su_tiles, sd_tiles), one per 128-col chunk.
        fast mode (layers 2-4): ALL means land in disjoint columns of ONE
        PSUM tile borrowed from ps_big (idle after the p-layers), staged by
        a single scalar copy -- without this the means rotate through the
        2-buffer ps_sm pool shared with the transposes, threading every
        su/sd chunk pair through a ~2.5us mean->copy->broadcast chain."""
        nch = width // 128
        su, sd = [], []
        if fast:
            psm_t = ps_big.tile([128, 1024], F32, tag="big1024")
            psm_all = psm_t[:, 0:2 * nch]
            for c in range(nch):
                for half in (0, 1):
                    base = half * 64
                    idx = 2 * c + half
                    nc.tensor.matmul(
                        psm_all[:, idx:idx + 1],
                        s_v[base:base + 64, c * 128:(c + 1) * 128],
                        inv64_col[base:base + 64, :],
                        start=True, stop=True, tile_position=(base, 0),
                        skip_group_check=(idx > 0))
            mcol_all = pipe2.tile([128, 8], F32, tag="mcolall")
            nc.scalar.activation(mcol_all[:, 0:2 * nch], psm_all,
                                 AF.Identity)
            for c in range(nch):
                for half, out_list in ((0, su), (1, sd)):
                    idx = 2 * c + half
                    bt = sbcast.tile([128, 128], FP16 if fp16 else F32,
                                     tag="sbcast" + ("h" if fp16 else "f"))
                    nc.scalar.activation(bt[:], ident[:], AF.Identity,
                                         bias=mcol_all[:, idx:idx + 1],
                                         scale=0.0)
                    out_list.append(bt)
            return su, sd
        for c in range(nch):
            sl = slice(c * 128, (c + 1) * 128)
            for half, out_list in ((0, su), (1, sd)):
                base = half * 64
                psm = ps_sm.tile([128, 1], F32, tag="small")
                nc.tensor.matmul(
                    psm[:], s_v[base:base + 64, sl],
                    inv64_col[base:base + 64, :],
                    start=True, stop=True,
                    tile_position=(base, 0),
                )
                # stage the mean column through the SCALAR engine, not DVE:
                # tiny DVE copies queue behind the 8.6us p-reduces on the
                # in-order DVE and stalled the whole s-chain ~7us.
                mcol = pipe2.tile([128, 1], F32, tag="mcol")
                nc.scalar.activation(mcol[:], psm[:], AF.Identity)
                bt = sbcast.tile([128, 128], FP16 if fp16 else F32,
                                 tag="sbcast" + ("h" if fp16 else "f"))
                nc.scalar.activation(bt[:], ident[:], AF.Identity,
                                     bias=mcol[:], scale=0.0)
                out_list.append(bt)
        return su, sd

    def s_transposes(s_v, width, lname):
        out = []
        for c in range(width // 128):
            sl = slice(c * 128, (c + 1) * 128)
            pst = ps_sm.tile([128, 128], F32, tag="small")
            nc.tensor.transpose(pst[:], s_v[:, sl], ident[:])
            svt = svtp.tile([128, 128], F32, tag="svT")
            nc.scalar.activation(svt[:], pst[:], AF.Identity)
            out.append(svt)
        return out

    def s_layer(lname, chunks, bias_tile):
        """chunks: (lhsT_ap, w_ap) pairs accumulated into one PSUM group.
        All matmuls are plain f32: f32r's truncation noise gets
        chaos-amplified through the 4-layer chain and the ill-conditioned
        logdet (measured ~100 absolute shift on HW); exact f32 at 4
        cycles/row is the price of correctness. Returns s_v [128,512] f32."""
        ps_t = ps_s.tile([128, 512], F32, tag="sx512")
        ps = ps_t[:]
        for idx, (lhsT, wap) in enumerate(chunks):
            nc.tensor.matmul(ps, lhsT, wap, start=(idx == 0), stop=False)
        nc.tensor.matmul(ps, ones_row[:], bias_tile[:],
                         start=False, stop=True)
        s_v = work.tile([128, 512], F32, tag=f"sv{lname}")
        nc.scalar.activation(s_v[:], ps, AF.Tanh)
        return s_v

    # layer 0: fin = 392 = su(128) sd(128) pu+pd(8) sv(128)
    sv0T = s_transposes(s_v0, 128, "0")
    su0, sd0 = s_means_bcast(s_v0, 128, "0", fp16=False)
    s_v1 = s_layer(
        "1",
        [(sv0T[0][:], WT0[:, 1536:2048]),
         (su0[0][:], WT0[:, 0:512]), (sd0[0][:], WT0[:, 512:1024]),
         (pm0[0:8, :], WT0[0:8, 1024:1536])],
        BT["V0_b"],
    )

    # layers 1, 2, after: fin = 1664 = su(512) sd(512) pu+pd(128) sv(512)
    # Vw chunk c occupies WT[:, c*512:(c+1)*512]; rows: su c0-3, sd c4-7,
    # pm c8, sv c9-12.
    def big_s_layer(lname, wth, wtf, bias_tile, s_v, pm_cum):
        svT = s_transposes(s_v, 512, lname)
        su, sd = s_means_bcast(s_v, 512, lname, fp16=(wth is not None),
                                fast=True)
        if wth is not None:
            # pm_cum enters its fp16 chunk rounded once per layer
            pmh = work.tile([128, 128], FP16, tag=f"pmh{lname}")
            nc.scalar.activation(pmh[:], pm_cum[:], AF.Identity)

        # chunk ORDER within the PSUM accumulation group is free; put the
        # transpose + pm chunks (ready ~1us after the tanh) first so the
        # means' psm->mcol->broadcast latency hides behind them.
        # pm LAST: its reduce is the slowest input (gated on the full
        # previous p-layer); everything else is ready within ~1us.
        chunks = []
        if wth is not None:
            for c in range(4):
                chunks.append((svT[c][:], wtf[:, c * 512:(c + 1) * 512]))
            for c in range(4):
                chunks.append((su[c][:], wth[:, c * 512:(c + 1) * 512]))
            for c in range(4):
                chunks.append((sd[c][:], wth[:, (4 + c) * 512:(5 + c) * 512]))
            # pu rows 1024:1088 / pd 1088:1152 are contiguous in Vw; pm_cum
            # holds pu at partitions 0:64, pd at 64:128 -- one K=128 chunk.
            chunks.append((pmh[:], wth[:, 8 * 512:9 * 512]))
        else:
            for c in range(4):
                chunks.append((svT[c][:], wtf[:, (9 + c) * 512:(10 + c) * 512]))
            for c in range(4):
                chunks.append((su[c][:], wtf[:, c * 512:(c + 1) * 512]))
            for c in range(4):
                chunks.append((sd[c][:], wtf[:, (4 + c) * 512:(5 + c) * 512]))
            chunks.append((pm_cum[:], wtf[:, 8 * 512:9 * 512]))
        return s_layer(lname, chunks, bias_tile)

    if stage == "s1" and dbg_d is not None:
        sv1f = work.tile([128, 512], F32, tag="sv1f")
        nc.scalar.activation(sv1f[:], s_v1[:], AF.Identity)
        dma(dbg_d[:], sv1f[:])
        dbg_out(s_v1[0:1, 0:1])
        for p in reversed(list(ctx_pools.values())):
            p.release()
        return

    t2 = p_layer(1, [(t1, 64)])
    pm1 = pm_part(t1, "1")
    s_v2 = big_s_layer("2", WTbigh["V1_w"], WTbig["V1_w"], BT["V1_b"], s_v1, pm1)
    # t3 reuses pT2_0's SBUF slot (pT2_0 is dead after layer 0 + its reduce)
    t3 = p_layer(2, [(t2, 64), (t1, 64)], out_tag="pT2_0")
    pm2 = pm_part(t2, "2")
    # accumulate means in-place: pm1 += pm2 (after layer 2 consumed pm1)
    nc.vector.tensor_tensor(pm1[:], pm1[:], pm2[:], op=Alu.add)
    s_v3 = big_s_layer("3", WTbigh["V2_w"], WTbig["V2_w"], BT["V2_b"], s_v2, pm1)
    pm3 = pm_part(t3, "3")
    nc.vector.tensor_tensor(pm1[:], pm1[:], pm3[:], op=Alu.add)
    s_v4 = big_s_layer("4", None, WTbig["after_w"], BT["after_b"], s_v3, pm1)

    if stage == "s":
        dbg_out(s_v4[0:1, 0:1])
        for p in reversed(list(ctx_pools.values())):
            p.release()
        return

    # ---------------- heads ----------------
    sv4T = s_transposes(s_v4, 512, "4")

    def head_half(wkey, bkey):
        ps = ps_sm.tile([64, 256], F32, tag="small")
        base = 0 if wkey == "vhu_w" else 64
        wt = WTvh[wkey]
        for c in range(4):
            nc.tensor.matmul(ps[:], sv4T[c][:, base:base + 64],
                             wt[:, c * 256:(c + 1) * 256],
                             start=(c == 0), stop=False)
        nc.tensor.matmul(ps[:], ones_row[:, 0:64], BT[bkey][:],
                         start=False, stop=True)
        sh = work.tile([64, 256], F32, tag="sh" + wkey)
        nc.vector.tensor_copy(sh[:], ps[:])
        return sh

    shu = head_half("vhu_w", "vhu_b")
    shd = head_half("vhd_w", "vhd_b")

    def head_T(sh, nm):
        out = []
        for c in range(2):
            pst = ps_sm.tile([128, 128], F32, tag="small")
            nc.tensor.transpose(pst[0:128, 0:64],
                                sh[:, c * 128:(c + 1) * 128],
                                ident[0:64, 0:64])
            ht = svtp.tile([128, 64], F32, tag="ht")
            nc.vector.tensor_copy(ht[:], pst[0:128, 0:64])
            out.append(ht)
        return out

    shuT = head_T(shu, "u")
    shdT = head_T(shd, "d")

    ps_A = ps_sm.tile([128, 64], F32, tag="small")
    for c in range(2):
        nc.tensor.matmul(ps_A[0:64, :], shuT[c][:],
                         WTw["wu_w"][:, c * 64:(c + 1) * 64],
                         start=(c == 0), stop=False, tile_position=(0, 0))
    nc.tensor.matmul(ps_A[0:64, :], ones_row[:, 0:64], BT["wu_b"][:],
                     start=False, stop=True, tile_position=(0, 0))
    for c in range(2):
        nc.tensor.matmul(ps_A[64:128, :], shdT[c][:],
                         WTw["wd_w"][:, c * 64:(c + 1) * 64],
                         start=(c == 0), stop=False, tile_position=(0, 64))
    nc.tensor.matmul(ps_A[64:128, :], ones_row[:, 0:64], BT["wd_b"][:],
                     start=False, stop=True, tile_position=(0, 64))

    # orb = s_w * (sum_j exp(-|ra|)) row-scale; stacked [A_u; A_d]
    A_sb = work.tile([128, 64], F32, tag="A_sb")
    nc.vector.tensor_scalar_mul(A_sb[:], ps_A[:], e_col[:])

    if stage == "heads":
        dbg_out(A_sb[0:1, 0:1])
        for p in reversed(list(ctx_pools.values())):
            p.release()
        return

    # warm the Abs/Ln activation tables now: the scalar engine is idle for
    # the whole ~90us LU phase, so the two ~1.3us ACT_TABLE_LOADs happen in
    # its shadow instead of on the logdet tail's critical path. Reading
    # A_sb (not a const) pins these AFTER the heads: the scheduler would
    # otherwise hoist them to t~8us where the p/s-phase Tanh/Identity/Exp
    # loads evict the tables again.
    warm = const.tile([1, 1], F32, tag="warm")
    nc.scalar.activation(warm[:], A_sb[0:1, 0:1], AF.Abs)
    nc.scalar.activation(warm[:], warm[:], AF.Ln)

    # ---------------- stacked unpivoted LU ----------------
    # Per step ONE matmul broadcasts the pivot row of each half to all
    # partitions of that half, straight into PSUM; the rhs is narrowed to
    # A[:, k:] (only live columns). The one-hot lhsT Cb (Cb[i,p] = 1 iff
    # i == k+64*(p>=64)) is built JUST-IN-TIME one step ahead with two DVE
    # tensor_scalar ops (ones64 * ident column) into a double-buffered
    # tile: those fill DVE's idle window while it waits for the matmul, so
    # they cost nothing. Then on DVE:
    #   r  = 1/ub[:,0]                      (per-partition u_kk broadcast)
    #   m  = (A[:,k] * r) * negmask[:,k]    (masked negated multipliers)
    #   A[:, k+1:] = (ub[:,1:] * m) + A     (fused rank-1 update, one op)
    # Rows <= k keep exact-zero multipliers so finished U rows are never
    # corrupted; diag(U) is read off the final A afterwards.
    def build_cb(k):
        # built on the SCALAR engine (idle during the LU) via the
        # bias-broadcast trick, so the DVE chain never sees these ops
        cb = lu.tile([128, 128], F32, tag=f"cb{k % 2}")
        nc.scalar.activation(cb[:, 0:64], ones64[:], AF.Identity,
                             bias=ident[:, k:k + 1], scale=0.0)
        nc.scalar.activation(cb[:, 64:128], ones64[:], AF.Identity,
                             bias=ident[:, 64 + k:65 + k], scale=0.0)
        return cb

    n_lu = {"lu16": 16}.get(stage, 63)
    cb = build_cb(0)
    for k in range(n_lu):
        w = 64 - k
        ps_ub = ps_lu.tile([128, 64], F32, tag="ub")
        # NOTE f32r here fails BIR verification: every producer writing A_sb
        # would have to round its output to f32r (changing LU numerics), so
        # the broadcast matmul stays plain f32 (2-pass at M=128, ~640ns).
        nc.tensor.matmul(ps_ub[:, 0:w], cb[:], A_sb[:, k:64],
                         start=True, stop=True)
        if k + 1 < n_lu:
            cb = build_cb(k + 1)
        rcol = lu.tile([128, 1], F32, tag="rcol")
        nc.vector.reciprocal(rcol[:], ps_ub[:, 0:1])
        m = lu.tile([128, 1], F32, tag="m")
        nc.vector.scalar_tensor_tensor(m[:], A_sb[:, k:k + 1], rcol[:],
                                       negmask[:, k:k + 1],
                                       op0=Alu.mult, op1=Alu.mult)
        nc.vector.scalar_tensor_tensor(
            A_sb[:, k + 1:64], ps_ub[:, 1:w], m[:], A_sb[:, k + 1:64],
            op0=Alu.mult, op1=Alu.add)

    if stage == "lu16":
        dbg_out(A_sb[0:1, 0:1])
        for p in reversed(list(ctx_pools.values())):
            p.release()
        return

    # logdet = sum_p ln|diag| ; diag[p] = A[p, p%64] via masked reduce
    prod = work.tile([128, 64], F32, tag="prod")
    dcol = work.tile([128, 1], F32, tag="dcol")
    nc.vector.tensor_tensor(prod[:], A_sb[:], istack[:], op=Alu.mult)
    nc.vector.reduce_sum(dcol[:], prod[:], axis=mybir.AxisListType.X)
    dabs = work.tile([128, 1], F32, tag="dabs")
    nc.scalar.activation(dabs[:], dcol[:], AF.Abs)
    dln = work.tile([128, 1], F32, tag="dln")
    nc.scalar.activation(dln[:], dabs[:], AF.Ln)
    # cross-partition sum: transpose the column to a row, reduce along free
    ps_out = ps_sm.tile([1, 128], F32, tag="small")
    nc.tensor.transpose(ps_out[:], dln[:], ident[:])
    s_row = work.tile([1, 128], F32, tag="s_row")
    nc.vector.tensor_copy(s_row[:], ps_out[:])
    tot = work.tile([1, 1], F32, tag="tot")
    nc.vector.reduce_sum(tot[:], s_row[:], axis=mybir.AxisListType.X)
    dma(out_d[:], tot[:])

    for p in reversed(list(ctx_pools.values())):
        p.release()


_NC_CACHE = {}


def build_nc():
    if "nc" in _NC_CACHE:
        return _NC_CACHE["nc"]
    import os
    nc = bacc.Bacc("TRN2", target_bir_lowering=False, debug=False)
    ins = {}
    for name, shape in INPUT_SPECS:
        ins[name] = nc.dram_tensor(name, list(shape), F32,
                                   kind="ExternalInput").ap()
    out_d = nc.dram_tensor("out", [1, 1], F32, kind="ExternalOutput").ap()
    dbg_d = None
    if os.environ.get("KSTAGE", "full") in ("s1", "pm0"):
        dbg_d = nc.dram_tensor("dbgout", [128, 512], F32,
                               kind="ExternalOutput").ap()
    with tile.TileContext(nc) as tc:
        _program(tc, nc, ins, out_d, dbg_d)
    nc.compile()
    _NC_CACHE["nc"] = nc
    return nc


def kernel(**inputs) -> np.ndarray:
    from concourse.bass_utils import run_bass_kernel_spmd

    nc = build_nc()
    in_map = {name: np.ascontiguousarray(np.asarray(inputs[name],
                                                    dtype=np.float32))
              for name, _ in INPUT_SPECS}
    in_maps = [in_map for _ in range(8)]
    res = run_bass_kernel_spmd(nc, in_maps, core_ids=list(range(8)))
    out = res.results[0]["out"]
    return np.float32(out.reshape(())[()])


# revision 63
# speedup vs baseline: 1.0186x; 1.0184x over previous
"""Trainium2 Bass kernel for the FermiNet-style single-configuration ansatz.

Computes log|psi| = logdet(orb_u) + logdet(orb_d) for one electron
configuration. The whole forward runs replicated on 8 NeuronCores (the
problem is tiny; inter-core collectives have a ~7-20us latency floor that
dwarfs the ~1 GFLOP of compute, so replication is the fastest correct
distribution) and core 0's scalar output is returned.

Layout choices (see inline comments):
  - p-tensor kept transposed+doubled: pT2[q, j*64 + i_local], q<64 = feature g
    for spin-up electrons (i<64), q>=64 = feature g for spin-down. This makes
    the per-pair feature matmul a K=64 contraction over partitions, lets
    spin-up/down run concurrently in separate PE array quadrants
    (tile_position), and makes the i-mean a free-dim segmented reduce.
  - residuals p2 = t2 + t1 are never materialized; the matmul and the means
    distribute over the sum (tanh outputs t_l are kept separately).
  - p-mean contributions to the s-layers use ONE cumulative [128,128] tile
    (DVE adds between layers) instead of repeated Vw chunks.
  - s-layer weights are all prefetched to SBUF right after the p-tensor
    build (descriptor-striped big DMAs), so s-layer matmuls never stall.
  - biases along the free dim are added as rank-1 (ones x b) matmuls into the
    same PSUM accumulation group.
  - logdet via unpivoted rank-1 Gaussian elimination on the stacked [A_u;A_d]
    [128,64] tile. Per step ONE K=128 one-hot matmul broadcasts the pivot
    row of both halves straight to PSUM, then 3 DVE ops (reciprocal /
    multiplier / fused scalar_tensor_tensor rank-1 update). The one-hot
    lhsT is built just-in-time one step ahead on the otherwise-idle SCALAR
    engine (bias-broadcast trick), so neither PE nor DVE pays for it.
    Unpivoted LU is stable here (growth factor ~700, logdet error ~6e-3 in
    f32 vs 2e-2 rtol). diag(U) is read off the final A at the end.

Hard-won platform notes (cost ~6 HW iterations to learn):
  - matmul PE time scales with OUT FREE SIZE (N) x cycles/row(rhs dtype),
    NOT with K; f32 runs as 2 half-speed passes when M > 64.
  - bulk GPSIMD ucode is poison: consumers of ANY gpsimd output wait for
    the engine's full queue DRAIN (a 16us CC build stalled all DMAs 15us).
  - stride-0 (broadcast_to) DMA APs fail neuronxcc codegen.
  - f32r matmuls require every PRODUCER of their operands to round to
    f32r (BIR verifier), and still hit an ISA check failure here.
  - tensor_tensor_reduce crashes the exec unit on TRN2 HW (sim-only op).
  - act tables: ~6-7 live; warm Abs/Ln AFTER the last Tanh/Identity use
    (pin the warm-up late via a data dependency or the scheduler hoists it).
"""

import numpy as np

import concourse.bass as bass
import concourse.bacc as bacc
import concourse.mybir as mybir
import concourse.tile as tile
from concourse import bass_isa, masks

F32 = mybir.dt.float32
F32R = mybir.dt.float32r
FP16 = mybir.dt.float16
AF = mybir.ActivationFunctionType
Alu = mybir.AluOpType

NE, NA, NSV, NPV, NU = 128, 32, 512, 64, 64

INPUT_SPECS = [
    ("r", (128, 3)), ("a", (32, 3)),
    ("V0_w", (392, 512)), ("V0_b", (512,)),
    ("V1_w", (1664, 512)), ("V1_b", (512,)),
    ("V2_w", (1664, 512)), ("V2_b", (512,)),
    ("W0_w", (4, 64)), ("W0_b", (64,)),
    ("W1_w", (64, 64)), ("W1_b", (64,)),
    ("W2_w", (64, 64)), ("W2_b", (64,)),
    ("after_w", (1664, 512)), ("after_b", (512,)),
    ("vhu_w", (512, 256)), ("vhu_b", (256,)),
    ("vhd_w", (512, 256)), ("vhd_b", (256,)),
    ("wu_w", (256, 64)), ("wu_b", (64,)),
    ("wd_w", (256, 64)), ("wd_b", (64,)),
]


def _r(ap):
    return ap.bitcast(F32R)


def _program(tc, nc, ins, out_d, dbg_d=None):
    import os
    stage = os.environ.get("KSTAGE", "full")
    ctx_pools = {}

    def pool(name, bufs, space="SBUF"):
        if name not in ctx_pools:
            ctx_pools[name] = tc.alloc_tile_pool(name=name, bufs=bufs,
                                                 space=space)
        return ctx_pools[name]

    const = pool("const", 1)
    work = pool("work", 1)
    pipe2 = pool("pipe2", 2)
    sbcast = pool("sbcast", 8)
    svtp = pool("svtp", 4)
    big = pool("big", 1)
    wpre = pool("wpre", 1)
    wstream = pool("wstream", 4)
    lu = pool("lu", 3)
    ps_big = pool("ps_big", 2, space="PSUM")
    ps_sm = pool("ps_sm", 2, space="PSUM")
    ps_lu = pool("ps_lu", 1, space="PSUM")
    ps_s = pool("ps_sx", 1, space="PSUM")

    dma = nc.sync.dma_start

    # ---------------- constants ----------------
    ident = const.tile([128, 128], F32, tag="ident")
    masks.make_identity(nc, ident[:])
    ones_row = const.tile([1, 128], F32, tag="ones_row")
    nc.gpsimd.memset(ones_row[:], 1.0)
    inv64_col = const.tile([128, 1], F32, tag="inv64")
    nc.gpsimd.memset(inv64_col[:], 1.0 / 64.0)

    # LU strict-lower mask (negated): negmask[p, k] = -1 iff (p % 64) > k.
    # affine_select indexes partitions view-relative (probed in sim), so the
    # same base works for both halves.
    negmask = const.tile([128, 64], F32, tag="negmask")
    nc.gpsimd.memset(negmask[:], -1.0)
    for half in range(2):
        nc.gpsimd.affine_select(
            out=negmask[half * 64:(half + 1) * 64, :],
            in_=negmask[half * 64:(half + 1) * 64, :],
            pattern=[[-1, 64]], compare_op=Alu.is_ge,
            fill=0.0, base=-1, channel_multiplier=1)

    # ---------------- geometry ----------------
    r_sb = work.tile([128, 3], F32, tag="r_sb")
    dma(r_sb[:], ins["r"][:])

    # rT4 = [r^T ; ones] as [4, 128]
    psr = ps_sm.tile([4, 128], F32, tag="small")
    nc.tensor.transpose(psr[0:3, :], r_sb[:], ident[:])
    rT4 = const.tile([4, 128], F32, tag="rT4")
    nc.gpsimd.memset(rT4[:], 1.0)  # row 3 stays ones
    nc.vector.tensor_copy(rT4[0:3, :], psr[0:3, :])

    def delta_rows(t, nj, val):
        """t[c, j*3+k] = val*(k == c) for c in 0..2; row 3 zeroed (DMA after).
        Compute-engine APs must start at partition 0/32/64/96, so build the
        delta pattern with one affine_select over all 4 rows. NOTE: keep
        total GPSIMD ucode work tiny -- every consumer of ANY gpsimd output
        waits for the engine's full queue drain."""
        nc.gpsimd.memset(t[:], val)
        nc.gpsimd.affine_select(
            out=t[:], in_=t[:], pattern=[[0, nj], [1, 3]],
            compare_op=Alu.is_equal, fill=0.0, base=0, channel_multiplier=-1)

    def into_row3(t, src_flat, width, tag, scale):
        st = pipe2.tile([1, width], F32, tag=tag)
        dma(st[:], src_flat)
        nc.scalar.mul(st[:], st[:], scale)
        dma(t[3:4, :], st[:])

    # Wra[4, 96]: ra = [r|1] @ Wra,  ra[i, j*3+c] = r[i,c] - a[j,c]
    # ra[i, j] = r[i] - a[j]
    Wra = const.tile([4, 3 * NA], F32, tag="Wra")
    delta_rows(Wra, NA, 1.0)
    into_row3(Wra, ins["a"][:].rearrange("a b -> (a b)"), 3 * NA, "nga", -1.0)

    ps_ra_t = ps_big.tile([128, 1024], F32, tag="big1024")
    ps_ra = ps_ra_t[:, 0:3 * NA]
    nc.tensor.matmul(ps_ra, rT4[:], Wra[:], start=True, stop=True)
    ra_sb = work.tile([128, 3 * NA], F32, tag="ra_sb")
    nc.vector.tensor_copy(ra_sb[:], ps_ra)
    ra2 = work.tile([128, 3 * NA], F32, tag="ra2")
    nc.scalar.square(ra2[:], ps_ra)
    ra_len2 = work.tile([128, NA], F32, tag="ra_len2")
    nc.vector.reduce_sum(
        ra_len2[:], ra2[:].rearrange("p (j c) -> p j c", c=3),
        axis=mybir.AxisListType.X,
    )
    ra_len = work.tile([128, NA], F32, tag="ra_len")
    nc.scalar.sqrt(ra_len[:], ra_len2[:])
    # e_col[i] = sum_j exp(-|r_i - a_j|)
    e_col = const.tile([128, 1], F32, tag="e_col")
    eexp = work.tile([128, NA], F32, tag="eexp")
    nc.scalar.activation(eexp[:], ra_len[:], AF.Exp, scale=-1.0,
                         accum_out=e_col[:])

    def dbg_out(src_ap):
        o = work.tile([1, 1], F32, tag="out_sb")
        nc.scalar.mul(o[:], src_ap, 1.0)
        dma(out_d[:], o[:])

    # s_v0 [128, 128]: interleaved [ra_x, ra_y, ra_z, |ra|] per atom
    s_v0 = work.tile([128, 128], F32, tag="s_v0")
    v4 = s_v0[:].rearrange("p (j k) -> p j k", k=4)
    nc.scalar.activation(v4[:, :, 0:3],
                         ra_sb[:].rearrange("p (j c) -> p j c", c=3),
                         AF.Identity)
    nc.scalar.activation(v4[:, :, 3:4],
                         ra_len[:].rearrange("p (j k) -> p j k", k=1),
                         AF.Identity)

    # rr: Wrr[4, 384], rr = [r|1] @ Wrr, rr[i, j*3+c] = r[i,c] - r[j,c]
    # reference convention: rr[i, j] = r[j] - r[i]
    Wrr = const.tile([4, 3 * NE], F32, tag="Wrr")
    delta_rows(Wrr, NE, -1.0)
    into_row3(Wrr, ins["r"][:].rearrange("a b -> (a b)"), 3 * NE, "ngr", 1.0)

    ps_rr_t = ps_big.tile([128, 1024], F32, tag="big1024")
    ps_rr = ps_rr_t[:, 0:3 * NE]
    nc.tensor.matmul(ps_rr, rT4[:], Wrr[:], start=True, stop=True)
    rr_sb = work.tile([128, 3 * NE], F32, tag="rr_sb")
    nc.vector.tensor_copy(rr_sb[:], ps_rr)
    rr2 = work.tile([128, 3 * NE], F32, tag="rr2")
    nc.scalar.square(rr2[:], ps_rr)
    rr_len2 = work.tile([128, NE], F32, tag="rr_len2")
    nc.vector.reduce_sum(
        rr_len2[:], rr2[:].rearrange("p (j c) -> p j c", c=3),
        axis=mybir.AxisListType.X,
    )
    rr_len = work.tile([128, NE], F32, tag="rr_len")
    nc.scalar.sqrt(rr_len[:], rr_len2[:])  # diagonal is exactly 0

    if stage == "geom":
        dbg_out(e_col[0:1, :])
        for p in reversed(list(ctx_pools.values())):
            p.release()
        return

    # ---------------- pT2_0: p_v0 in transposed-doubled layout ----------------
    # pT2_0[g, j*64+il] = p_v0[il, j, g] (u half, partitions 0..3)
    # pT2_0[64+g, ...] = p_v0[64+il, j, g] (d half, partitions 64..67)
    pT2_0 = big.tile([128, 8192], FP16, tag="pT2_0")
    for g in range(4):
        if g < 3:
            # TensorE transpose silently no-ops the transpose for strided
            # inputs on HW (sim transposes) -- stage through a contiguous tile
            cont = pipe2.tile([128, 128], F32, tag="contg")
            nc.vector.tensor_copy(
                cont[:], rr_sb[:].rearrange("p (j c) -> p j c", c=3)[:, :, g])
            src = cont[:]
        else:
            src = rr_len[:]
        pst = ps_sm.tile([128, 128], F32, tag="small")
        nc.tensor.transpose(pst[:], src, ident[:])  # pst[j, i] = p0[i, j, g]
        pstc = pipe2.tile([128, 128], FP16, tag="p0T")
        nc.vector.tensor_copy(pstc[:], pst[:])
        du = pT2_0[g:g + 1, :].rearrange("p (j i) -> p j i", i=64)
        dd = pT2_0[64 + g:65 + g, :].rearrange("p (j i) -> p j i", i=64)
        # d-half data also at partitions 4..7 so layer-0's p-mean chunk can
        # be a single base-0 K=8 matmul (a tile_position'd matmul cannot
        # share an accumulation group with full-K ones on HW)
        dd2 = pT2_0[4 + g:5 + g, :].rearrange("p (j i) -> p j i", i=64)
        dma(du[:], pstc[:, 0:64])
        dma(dd[:], pstc[:, 64:128])
        dma(dd2[:], pstc[:, 64:128])

    # ---------------- p-layer weights (doubled to both partition halves) ----
    Wp, Wpb, Kp = [], [], [4, 64, 64]
    for l, (wn, bn) in enumerate([("W0_w", "W0_b"), ("W1_w", "W1_b"),
                                  ("W2_w", "W2_b")]):
        K = Kp[l]
        wstage = pipe2.tile([64, 64], F32, tag="wstage")
        dma(wstage[0:K, :], ins[wn][:])
        wt = const.tile([128, 64], FP16, tag=f"wp{l}")
        nc.vector.tensor_copy(wt[0:K, :], wstage[0:K, :])
        nc.vector.tensor_copy(wt[64:64 + K, :], wstage[0:K, :])
        bc = const.tile([128, 1], F32, tag=f"wpb{l}")
        dma(bc[0:64, :], ins[bn][:].rearrange("(a k) -> a k", k=1))
        dma(bc[64:128, :], ins[bn][:].rearrange("(a k) -> a k", k=1))
        Wp.append(wt)
        Wpb.append(bc)

    # ---------------- s-weight prefetch ----------------
    # All s-layer / head weights staged to SBUF now: the DMA descriptors
    # stripe across the 16 queues and land well before the s-layers start,
    # so no matmul ever waits on HBM. Emitted AFTER the pT2_0 build DMAs so
    # those small critical transfers aren't stuck behind 3.4MB of weights.
    # V0_w chunk rows: su 0:128, sd 128:256, pm 256:264 (K=8), sv 264:392.
    WT0 = wpre.tile([128, 4 * 512], F32, tag="WT0")
    dma(WT0[:, 0:512], ins["V0_w"][0:128, :])
    dma(WT0[:, 512:1024], ins["V0_w"][128:256, :])
    dma(WT0[0:8, 1024:1536], ins["V0_w"][256:264, :])
    dma(WT0[:, 1536:2048], ins["V0_w"][264:392, :])

    # su/sd/pm weight rows (0:1152) are consumed by fp16 matmuls (their
    # lhsT carries smooth MEAN signals; fp16's 2.4e-4 rel rounding is far
    # below the tanh-chain's noise floor, and fp16 runs 1 cyc/row single
    # pass vs f32's two half-speed passes). sv rows (1152:1664) stay f32:
    # they carry the raw activations that dominate the det sensitivity.
    WTbig = {}
    WTbigh = {}
    for wk in ("V1_w", "V2_w"):
        th = wpre.tile([128, 9 * 512], FP16, tag=f"WTh_{wk}")
        for c in range(9):
            wv = wstream.tile([128, 512], F32, tag="vw")
            dma(wv[:], ins[wk][c * 128:(c + 1) * 128, :])
            nc.scalar.activation(th[:, c * 512:(c + 1) * 512], wv[:],
                                 AF.Identity)
        WTbigh[wk] = th
        t = wpre.tile([128, 4 * 512], F32, tag=f"WT_{wk}")
        dma(t[:].rearrange("p (c n) -> p c n", n=512),
            ins[wk][1152:1664, :].rearrange("(c p) n -> p c n", p=128))
        WTbig[wk] = t
    # layer 4 (after_w) feeds the heads/determinant directly: keep it exact
    tf = wpre.tile([128, 13 * 512], F32, tag="WT_after_w")
    dma(tf[:].rearrange("p (c n) -> p c n", n=512),
        ins["after_w"][:].rearrange("(c p) n -> p c n", p=128))
    WTbig["after_w"] = tf

    WTvh = {}
    for wk in ("vhu_w", "vhd_w"):
        t = wpre.tile([128, 4 * 256], F32, tag=f"WT_{wk}")
        dma(t[:].rearrange("p (c n) -> p c n", n=256),
            ins[wk][:].rearrange("(c p) n -> p c n", p=128))
        WTvh[wk] = t
    WTw = {}
    for wk in ("wu_w", "wd_w"):
        t = wpre.tile([128, 2 * 64], F32, tag=f"WT_{wk}")
        dma(t[:].rearrange("p (c n) -> p c n", n=64),
            ins[wk][:].rearrange("(c p) n -> p c n", p=128))
        WTw[wk] = t

    BT = {}
    for bk, w in (("V0_b", 512), ("V1_b", 512), ("V2_b", 512),
                  ("after_b", 512), ("vhu_b", 256), ("vhd_b", 256),
                  ("wu_b", 64), ("wd_b", 64)):
        t = wpre.tile([1, w], F32, tag=f"BT_{bk}")
        dma(t[:], ins[bk][:].rearrange("(k a) -> k a", k=1))
        BT[bk] = t

    # ones64: in0 operand for the just-in-time one-hot build in the LU loop
    ones64 = const.tile([128, 64], F32, tag="ones64")
    nc.gpsimd.memset(ones64[:], 1.0)

    # istack[p, j] = 1 iff p%64 == j  (diag extraction mask for the end)
    istack = const.tile([128, 64], F32, tag="istack")
    nc.vector.tensor_copy(istack[0:64, :], ident[0:64, 0:64])
    nc.vector.tensor_copy(istack[64:128, :], ident[64:128, 64:128])

    # ---------------- p-layers ----------------
    # t_{l+1} = tanh(W_l^T applied to p_v_l); p_v residuals kept distributed.
    t_tiles = []

    def p_layer(l, rhs_list, out_tag=None):
        """rhs_list: list of (tile, K) contributions summed pre-tanh.
        Two 512-col chunks share one [128,1024] PSUM tile (2 banks) so the
        tanh runs as ONE activation per pair: the ~370ns per-op ACT
        overhead dominates the 512-element data time, so halving the op
        count saves ~9us across the three layers."""
        out_t = big.tile([128, 8192], FP16, tag=out_tag or f"t{l + 1}")
        wt, bc = Wp[l], Wpb[l]
        for c2 in range(8):
            ps = ps_big.tile([128, 1024], F32, tag="big1024")
            n = len(rhs_list)
            for half in range(2):
                c = 2 * c2 + half
                sl = slice(c * 512, (c + 1) * 512)
                pssl = slice(half * 512, (half + 1) * 512)
                for idx, (src, K) in enumerate(rhs_list):
                    # independent accumulation group per psum region; the
                    # half-0 u-area brackets the sim's per-tensor group,
                    # the other three areas skip the (bank-global) check
                    st, sp = idx == 0, idx == n - 1
                    nc.tensor.matmul(ps[0:64, pssl], wt[0:K, :],
                                     src[0:K, sl],
                                     start=st, stop=sp, tile_position=(0, 0),
                                     skip_group_check=(half == 1))
                    # skip_group_check: the sim's zero-region tracking is
                    # bank-global, but disjoint groups are sound
                    # (per-element has_written bits); verified numerically.
                    nc.tensor.matmul(ps[64:128, pssl], wt[64:64 + K, :],
                                     src[64:64 + K, sl],
                                     start=st, stop=sp,
                                     tile_position=(64, 64),
                                     skip_group_check=True)
            nc.scalar.activation(out_t[:, c2 * 1024:(c2 + 1) * 1024], ps[:],
                                 AF.Tanh, bias=bc[:])
        t_tiles.append(out_t)
        return out_t

    t1 = p_layer(0, [(pT2_0, 4)])

    # ---------------- p means (cumulative, scaled 1/64) ----------------
    # red_l[q, j] = sum_il t_l[q, j*64+il]; pmean chunks feed s-matmul lhsT.
    def p_reduce(src, tag):
        # quarter-split: each 2.15us piece starts once its quarter of the
        # tanh output lands, instead of one 8.6us op gated on the full tile
        red = work.tile([128, 128], F32, tag=tag)
        for q in range(4):
            nc.vector.reduce_sum(
                red[:, q * 32:(q + 1) * 32],
                src[:, q * 2048:(q + 1) * 2048].rearrange(
                    "p (j i) -> p j i", i=64),
                axis=mybir.AxisListType.X,
            )
        return red

    # pT2_0 rows 0-3 = u features, rows 4-7 = d (duplicated); one K=8 block
    red0 = work.tile([128, 128], F32, tag="red0")
    pm0 = work.tile([128, 128], F32, tag="pm0")
    for q in range(4):
        nc.vector.reduce_sum(
            red0[0:8, q * 32:(q + 1) * 32],
            pT2_0[0:8, q * 2048:(q + 1) * 2048].rearrange(
                "p (j i) -> p j i", i=64),
            axis=mybir.AxisListType.X,
        )
    nc.scalar.activation(pm0[0:8, :], red0[0:8, :],
                         AF.Identity, scale=1.0 / 64.0)

    def pm_part(t, tag):
        red = p_reduce(t, "red" + tag)
        pm = work.tile([128, 128], F32, tag="pm" + tag)
        nc.scalar.activation(pm[:], red[:], AF.Identity, scale=1.0 / 64.0)
        return pm

    # ---------------- s-layers ----------------
    # Emission interleaves the s-chain INTO the p-chain: s_v1 only needs
    # s_v0+pm0, s_v2 needs pm1 (t1's reduce), etc. -- so the scheduler can
    # slot s-layer PE chunks into the p-phase's PE gaps.
    def s_means_bcast(s_v, width, lname, fp16=True, fast=False):
        """Column-mean of the u/d row-halves of s_v, broadcast to [128,128]
        lhsT tiles. Returns (su_tiles, sd_tiles), one per 128-col chunk.
        fast mode (layers 2-4): ALL means land in disjoint columns of ONE
        PSUM tile borrowed from ps_big (idle after the p-layers), staged by
        a single scalar copy -- without this the means rotate through the
        2-buffer ps_sm pool shared with the transposes, threading every
        su/sd chunk pair through a ~2.5us mean->copy->broadcast chain."""
        nch = width // 128
        su, sd = [], []
        if fast:
            psm_t = ps_big.tile([128, 1024], F32, tag="big1024")
            psm_all = psm_t[:, 0:2 * nch]
            for c in range(nch):
                for half in (0, 1):
                    base = half * 64
                    idx = 2 * c + half
                    nc.tensor.matmul(
                        psm_all[:, idx:idx + 1],
                        s_v[base:base + 64, c * 128:(c + 1) * 128],
                        inv64_col[base:base + 64, :],
                        start=True, stop=True, tile_position=(base, 0),
                        skip_group_check=(idx > 0))
            mcol_all = pipe2.tile([128, 8], F32, tag="mcolall")
            nc.scalar.activation(mcol_all[:, 0:2 * nch], psm_all,
                                 AF.Identity)
            for c in range(nch):
                for half, out_list in ((0, su), (1, sd)):
                    idx = 2 * c + half
                    bt = sbcast.tile([128, 128], FP16 if fp16 else F32,
                                     tag="sbcast" + ("h" if fp16 else "f"))
                    nc.scalar.activation(bt[:], ident[:], AF.Identity,
                                         bias=mcol_all[:, idx:idx + 1],
                                         scale=0.0)
                    out_list.append(bt)
            return su, sd
        for c in range(nch):
            sl = slice(c * 128, (c + 1) * 128)
            for half, out_list in ((0, su), (1, sd)):
                base = half * 64
                psm = ps_sm.tile([128, 1], F32, tag="small")
                nc.tensor.matmul(
                    psm[:], s_v[base:base + 64, sl],
                    inv64_col[base:base + 64, :],
                    start=True, stop=True,
                    tile_position=(base, 0),
                )
                # stage the mean column through the SCALAR engine, not DVE:
                # tiny DVE copies queue behind the 8.6us p-reduces on the
                # in-order DVE and stalled the whole s-chain ~7us.
                mcol = pipe2.tile([128, 1], F32, tag="mcol")
                nc.scalar.activation(mcol[:], psm[:], AF.Identity)
                bt = sbcast.tile([128, 128], FP16 if fp16 else F32,
                                 tag="sbcast" + ("h" if fp16 else "f"))
                nc.scalar.activation(bt[:], ident[:], AF.Identity,
                                     bias=mcol[:], scale=0.0)
                out_list.append(bt)
        return su, sd

    def s_transposes(s_v, width, lname):
        out = []
        for c in range(width // 128):
            sl = slice(c * 128, (c + 1) * 128)
            pst = ps_sm.tile([128, 128], F32, tag="small")
            nc.tensor.transpose(pst[:], s_v[:, sl], ident[:])
            svt = svtp.tile([128, 128], F32, tag="svT")
            nc.scalar.activation(svt[:], pst[:], AF.Identity)
            out.append(svt)
        return out

    def s_layer(lname, chunks, bias_tile):
        """chunks: (lhsT_ap, w_ap) pairs accumulated into one PSUM group.
        All matmuls are plain f32: f32r's truncation noise gets
        chaos-amplified through the 4-layer chain and the ill-conditioned
        logdet (measured ~100 absolute shift on HW); exact f32 at 4
        cycles/row is the price of correctness. Returns s_v [128,512] f32."""
        ps_t = ps_s.tile([128, 512], F32, tag="sx512")
        ps = ps_t[:]
        for idx, (lhsT, wap) in enumerate(chunks):
            nc.tensor.matmul(ps, lhsT, wap, start=(idx == 0), stop=False)
        nc.tensor.matmul(ps, ones_row[:], bias_tile[:],
                         start=False, stop=True)
        s_v = work.tile([128, 512], F32, tag=f"sv{lname}")
        nc.scalar.activation(s_v[:], ps, AF.Tanh)
        return s_v

    # layer 0: fin = 392 = su(128) sd(128) pu+pd(8) sv(128)
    sv0T = s_transposes(s_v0, 128, "0")
    su0, sd0 = s_means_bcast(s_v0, 128, "0", fp16=False)
    s_v1 = s_layer(
        "1",
        [(sv0T[0][:], WT0[:, 1536:2048]),
         (su0[0][:], WT0[:, 0:512]), (sd0[0][:], WT0[:, 512:1024]),
         (pm0[0:8, :], WT0[0:8, 1024:1536])],
        BT["V0_b"],
    )

    # layers 1, 2, after: fin = 1664 = su(512) sd(512) pu+pd(128) sv(512)
    # Vw chunk c occupies WT[:, c*512:(c+1)*512]; rows: su c0-3, sd c4-7,
    # pm c8, sv c9-12.
    def big_s_layer(lname, wth, wtf, bias_tile, s_v, pm_cum):
        svT = s_transposes(s_v, 512, lname)
        su, sd = s_means_bcast(s_v, 512, lname, fp16=(wth is not None),
                                fast=True)
        if wth is not None:
            # pm_cum enters its fp16 chunk rounded once per layer
            pmh = work.tile([128, 128], FP16, tag=f"pmh{lname}")
            nc.scalar.activation(pmh[:], pm_cum[:], AF.Identity)

        # chunk ORDER within the PSUM accumulation group is free; put the
        # transpose + pm chunks (ready ~1us after the tanh) first so the
        # means' psm->mcol->broadcast latency hides behind them.
        # pm LAST: its reduce is the slowest input (gated on the full
        # previous p-layer); everything else is ready within ~1us.
        chunks = []
        if wth is not None:
            for c in range(4):
                chunks.append((svT[c][:], wtf[:, c * 512:(c + 1) * 512]))
            for c in range(4):
                chunks.append((su[c][:], wth[:, c * 512:(c + 1) * 512]))
            for c in range(4):
                chunks.append((sd[c][:], wth[:, (4 + c) * 512:(5 + c) * 512]))
            # pu rows 1024:1088 / pd 1088:1152 are contiguous in Vw; pm_cum
            # holds pu at partitions 0:64, pd at 64:128 -- one K=128 chunk.
            chunks.append((pmh[:], wth[:, 8 * 512:9 * 512]))
        else:
            for c in range(4):
                chunks.append((svT[c][:], wtf[:, (9 + c) * 512:(10 + c) * 512]))
            for c in range(4):
                chunks.append((su[c][:], wtf[:, c * 512:(c + 1) * 512]))
            for c in range(4):
                chunks.append((sd[c][:], wtf[:, (4 + c) * 512:(5 + c) * 512]))
            chunks.append((pm_cum[:], wtf[:, 8 * 512:9 * 512]))
        return s_layer(lname, chunks, bias_tile)

    if stage == "s1" and dbg_d is not None:
        sv1f = work.tile([128, 512], F32, tag="sv1f")
        nc.scalar.activation(sv1f[:], s_v1[:], AF.Identity)
        dma(dbg_d[:], sv1f[:])
        dbg_out(s_v1[0:1, 0:1])
        for p in reversed(list(ctx_pools.values())):
            p.release()
        return

    t2 = p_layer(1, [(t1, 64)])
    pm1 = pm_part(t1, "1")
    s_v2 = big_s_layer("2", WTbigh["V1_w"], WTbig["V1_w"], BT["V1_b"], s_v1, pm1)
    # t3 reuses pT2_0's SBUF slot (pT2_0 is dead after layer 0 + its reduce)
    t3 = p_layer(2, [(t2, 64), (t1, 64)], out_tag="pT2_0")
    pm2 = pm_part(t2, "2")
    # accumulate means in-place: pm1 += pm2 (after layer 2 consumed pm1)
    nc.vector.tensor_tensor(pm1[:], pm1[:], pm2[:], op=Alu.add)
    s_v3 = big_s_layer("3", WTbigh["V2_w"], WTbig["V2_w"], BT["V2_b"], s_v2, pm1)
    pm3 = pm_part(t3, "3")
    nc.vector.tensor_tensor(pm1[:], pm1[:], pm3[:], op=Alu.add)
    s_v4 = big_s_layer("4", None, WTbig["after_w"], BT["after_b"], s_v3, pm1)

    if stage == "s":
        dbg_out(s_v4[0:1, 0:1])
        for p in reversed(list(ctx_pools.values())):
            p.release()
        return

    # ---------------- heads ----------------
    sv4T = s_transposes(s_v4, 512, "4")

    def head_half(wkey, bkey):
        ps = ps_sm.tile([64, 256], F32, tag="small")
        base = 0 if wkey == "vhu_w" else 64
        wt = WTvh[wkey]
        for c in range(4):
            nc.tensor.matmul(ps[:], sv4T[c][:, base:base + 64],
                             wt[:, c * 256:(c + 1) * 256],
                             start=(c == 0), stop=False)
        nc.tensor.matmul(ps[:], ones_row[:, 0:64], BT[bkey][:],
                         start=False, stop=True)
        sh = work.tile([64, 256], F32, tag="sh" + wkey)
        nc.vector.tensor_copy(sh[:], ps[:])
        return sh

    shu = head_half("vhu_w", "vhu_b")
    shd = head_half("vhd_w", "vhd_b")

    def head_T(sh, nm):
        out = []
        for c in range(2):
            pst = ps_sm.tile([128, 128], F32, tag="small")
            nc.tensor.transpose(pst[0:128, 0:64],
                                sh[:, c * 128:(c + 1) * 128],
                                ident[0:64, 0:64])
            ht = svtp.tile([128, 64], F32, tag="ht")
            nc.vector.tensor_copy(ht[:], pst[0:128, 0:64])
            out.append(ht)
        return out

    shuT = head_T(shu, "u")
    shdT = head_T(shd, "d")

    ps_A = ps_sm.tile([128, 64], F32, tag="small")
    for c in range(2):
        nc.tensor.matmul(ps_A[0:64, :], shuT[c][:],
                         WTw["wu_w"][:, c * 64:(c + 1) * 64],
                         start=(c == 0), stop=False, tile_position=(0, 0))
    nc.tensor.matmul(ps_A[0:64, :], ones_row[:, 0:64], BT["wu_b"][:],
                     start=False, stop=True, tile_position=(0, 0))
    for c in range(2):
        nc.tensor.matmul(ps_A[64:128, :], shdT[c][:],
                         WTw["wd_w"][:, c * 64:(c + 1) * 64],
                         start=(c == 0), stop=False, tile_position=(0, 64))
    nc.tensor.matmul(ps_A[64:128, :], ones_row[:, 0:64], BT["wd_b"][:],
                     start=False, stop=True, tile_position=(0, 64))

    # orb = s_w * (sum_j exp(-|ra|)) row-scale; stacked [A_u; A_d]
    A_sb = work.tile([128, 64], F32, tag="A_sb")
    nc.vector.tensor_scalar_mul(A_sb[:], ps_A[:], e_col[:])

    if stage == "heads":
        dbg_out(A_sb[0:1, 0:1])
        for p in reversed(list(ctx_pools.values())):
            p.release()
        return

    # warm the Abs/Ln activation tables now: the scalar engine is idle for
    # the whole ~90us LU phase, so the two ~1.3us ACT_TABLE_LOADs happen in
    # its shadow instead of on the logdet tail's critical path. Reading
    # A_sb (not a const) pins these AFTER the heads: the scheduler would
    # otherwise hoist them to t~8us where the p/s-phase Tanh/Identity/Exp
    # loads evict the tables again.
    warm = const.tile([1, 1], F32, tag="warm")
    nc.scalar.activation(warm[:], A_sb[0:1, 0:1], AF.Abs)
    nc.scalar.activation(warm[:], warm[:], AF.Ln)

    # ---------------- stacked unpivoted LU ----------------
    # Per step ONE matmul broadcasts the pivot row of each half to all
    # partitions of that half, straight into PSUM; the rhs is narrowed to
    # A[:, k:] (only live columns). The one-hot lhsT Cb (Cb[i,p] = 1 iff
    # i == k+64*(p>=64)) is built JUST-IN-TIME one step ahead with two DVE
    # tensor_scalar ops (ones64 * ident column) into a double-buffered
    # tile: those fill DVE's idle window while it waits for the matmul, so
    # they cost nothing. Then on DVE:
    #   r  = 1/ub[:,0]                      (per-partition u_kk broadcast)
    #   m  = (A[:,k] * r) * negmask[:,k]    (masked negated multipliers)
    #   A[:, k+1:] = (ub[:,1:] * m) + A     (fused rank-1 update, one op)
    # Rows <= k keep exact-zero multipliers so finished U rows are never
    # corrupted; diag(U) is read off the final A afterwards.
    def build_cb(k):
        # built on the SCALAR engine (idle during the LU) via the
        # bias-broadcast trick, so the DVE chain never sees these ops
        cb = lu.tile([128, 128], F32, tag=f"cb{k % 2}")
        nc.scalar.activation(cb[:, 0:64], ones64[:], AF.Identity,
                             bias=ident[:, k:k + 1], scale=0.0)
        nc.scalar.activation(cb[:, 64:128], ones64[:], AF.Identity,
                             bias=ident[:, 64 + k:65 + k], scale=0.0)
        return cb

    n_lu = {"lu16": 16}.get(stage, 63)
    cb = build_cb(0)
    for k in range(n_lu):
        w = 64 - k
        ps_ub = ps_lu.tile([128, 64], F32, tag="ub")
        # NOTE f32r here fails BIR verification: every producer writing A_sb
        # would have to round its output to f32r (changing LU numerics), so
        # the broadcast matmul stays plain f32 (2-pass at M=128, ~640ns).
        nc.tensor.matmul(ps_ub[:, 0:w], cb[:], A_sb[:, k:64],
                         start=True, stop=True)
        if k + 1 < n_lu:
            cb = build_cb(k + 1)
        rcol = lu.tile([128, 1], F32, tag="rcol")
        nc.vector.reciprocal(rcol[:], ps_ub[:, 0:1])
        m = lu.tile([128, 1], F32, tag="m")
        nc.vector.scalar_tensor_tensor(m[:], A_sb[:, k:k + 1], rcol[:],
                                       negmask[:, k:k + 1],
                                       op0=Alu.mult, op1=Alu.mult)
        nc.vector.scalar_tensor_tensor(
            A_sb[:, k + 1:64], ps_ub[:, 1:w], m[:], A_sb[:, k + 1:64],
            op0=Alu.mult, op1=Alu.add)

    if stage == "lu16":
        dbg_out(A_sb[0:1, 0:1])
        for p in reversed(list(ctx_pools.values())):
            p.release()
        return

    # logdet = sum_p ln|diag| ; diag[p] = A[p, p%64] via masked reduce
    prod = work.tile([128, 64], F32, tag="prod")
    dcol = work.tile([128, 1], F32, tag="dcol")
    nc.vector.tensor_tensor(prod[:], A_sb[:], istack[:], op=Alu.mult)
    nc.vector.reduce_sum(dcol[:], prod[:], axis=mybir.AxisListType.X)
    dabs = work.tile([128, 1], F32, tag="dabs")
    nc.scalar.activation(dabs[:], dcol[:], AF.Abs)
    dln = work.tile([128, 1], F32, tag="dln")
    nc.scalar.activation(dln[:], dabs[:], AF.Ln)
    # cross-partition sum: transpose the column to a row, reduce along free
    ps_out = ps_sm.tile([1, 128], F32, tag="small")
    nc.tensor.transpose(ps_out[:], dln[:], ident[:])
    s_row = work.tile([1, 128], F32, tag="s_row")
    nc.vector.tensor_copy(s_row[:], ps_out[:])
    tot = work.tile([1, 1], F32, tag="tot")
    nc.vector.reduce_sum(tot[:], s_row[:], axis=mybir.AxisListType.X)
    dma(out_d[:], tot[:])

    for p in reversed(list(ctx_pools.values())):
        p.release()


_NC_CACHE = {}


def build_nc():
    if "nc" in _NC_CACHE:
        return _NC_CACHE["nc"]
    import os
    nc = bacc.Bacc("TRN2", target_bir_lowering=False, debug=False)
    ins = {}
    for name, shape in INPUT_SPECS:
        ins[name] = nc.dram_tensor(name, list(shape), F32,
                                   kind="ExternalInput").ap()
    out_d = nc.dram_tensor("out", [1, 1], F32, kind="ExternalOutput").ap()
    dbg_d = None
    if os.environ.get("KSTAGE", "full") in ("s1", "pm0"):
        dbg_d = nc.dram_tensor("dbgout", [128, 512], F32,
                               kind="ExternalOutput").ap()
    with tile.TileContext(nc) as tc:
        _program(tc, nc, ins, out_d, dbg_d)
    nc.compile()
    _NC_CACHE["nc"] = nc
    return nc


def kernel(**inputs) -> np.ndarray:
    from concourse.bass_utils import run_bass_kernel_spmd

    nc = build_nc()
    in_map = {name: np.ascontiguousarray(np.asarray(inputs[name],
                                                    dtype=np.float32))
              for name, _ in INPUT_SPECS}
    in_maps = [in_map for _ in range(8)]
    res = run_bass_kernel_spmd(nc, in_maps, core_ids=list(range(8)))
    out = res.results[0]["out"]
    return np.float32(out.reshape(())[()])
